# revision 1
# baseline (speedup 1.0000x reference)
"""DBPNet Trainium2 kernel: 8-core data-parallel Bass/Tile implementation.

Scheme (validated by layout_sim.py):
  - batch-major state [32, N]: row = chan*16 + s (16 samples/core)
  - complex matmuls are "state-stationary": lhsT = combo(state) [2K-chunked, 32],
    rhs = replicated matrix stacks streamed as the moving operand (fp32r)
  - comboH (A^H-type):  top [s_r|s_i], bottom [s_i|-s_r]
  - comboN (C*s-type):  top [s_r|s_i], bottom [-s_i|s_r]
  - AAH = A A^H precomputed on host =>  Atx = arc + rho*c1*AAH*zmu,
    Ax = Atx - AAH*tmv  (A-matmuls only once per iteration)
  - CNN in (co*4+q, (b', l)) layout with block-diagonal weights
  - BatchNorm batch stats made exact across cores via AllReduce
"""
import numpy as np

B, Nv, Nt, F = 128, 512, 2048, 32
NCORE, BS = 8, 16
ITERS, ADMM = 5, 3
BN_EPS = 1e-5


# ---------------------------------------------------------------- host prep
def _host_prep(inputs):
    A = np.ascontiguousarray(np.asarray(inputs['A'], np.float32))
    Ar, Ai = A[0], A[1]
    Ac = Ar.astype(np.float64) + 1j * Ai.astype(np.float64)
    AAH = Ac @ Ac.conj().T
    AAHr = AAH.real.astype(np.float32)
    AAHi = AAH.imag.astype(np.float32)

    rhos = np.exp(np.asarray(inputs['log_rho'], np.float32)).astype(np.float32)
    epss = np.exp(np.asarray(inputs['log_eps'], np.float32)).astype(np.float32)

    minv_stacks, rho_to_idx, iter_minv_idx = [], {}, []
    for r in rhos:
        key = float(r)
        if key not in rho_to_idx:
            M = np.linalg.inv(AAH + key * np.eye(Nv))
            Mr = M.real.astype(np.float32)
            Mi = M.imag.astype(np.float32)
            minv_stacks.append(
                np.concatenate([Mr.T, Mi.T], 0).reshape(8, 128, 512)
                .transpose(1, 0, 2).copy())            # [128, 8, 512]
            rho_to_idx[key] = len(minv_stacks) - 1
        iter_minv_idx.append(rho_to_idx[float(r)])

    A1 = np.concatenate([Ar, Ai], 0)                    # [1024, 2048]
    AB = A1.reshape(8, 128, 2048).transpose(1, 0, 2).copy()   # [128, 8, 2048]
    AT1 = np.concatenate([Ar.T, Ai.T], 0)               # [4096, 512]
    ATD = AT1.reshape(32, 128, 512).copy()              # [32][128, 512]
    AAH1 = np.concatenate([AAHr.T, AAHi.T], 0)          # [1024, 512]
    AAHD = AAH1.reshape(8, 128, 512).transpose(1, 0, 2).copy()  # [128, 8, 512]

    w1 = np.asarray(inputs['conv1_w'], np.float32)
    w2 = np.asarray(inputs['conv2_w'], np.float32)
    wf = np.asarray(inputs['convf_w'], np.float32)
    W1 = np.zeros((128, 128), np.float32)
    for dl in range(3):
        for ci in range(2):
            for q in range(4):
                W1[dl * 8 + ci * 4 + q, np.arange(F) * 4 + q] = w1[:, ci, dl]
    W2 = np.zeros((3, 128, 128), np.float32)
    WFm = np.zeros((3, 128, 8), np.float32)
    for dl in range(3):
        for ci in range(F):
            for q in range(4):
                W2[dl, ci * 4 + q, np.arange(F) * 4 + q] = w2[:, ci, dl]
                WFm[dl, ci * 4 + q, np.arange(2) * 4 + q] = wf[:, ci, dl]

    onesel = np.zeros((128, 32), np.float32)
    selback = np.zeros((128, 128), np.float32)   # rows 0-31 used
    for co in range(32):
        for q in range(4):
            onesel[co * 4 + q, co] = 1.0
            selback[co, co * 4 + q] = 1.0
    ident32 = np.zeros((128, 32), np.float32)
    ident32[:32, :32] = np.eye(32, dtype=np.float32)
    ones11 = np.zeros((128, 1), np.float32)
    ones11[0, 0] = 1.0

    # WTS pack [128, 128 + 384 + 24 + 32 + 128 + 32 + 1]
    WTS = np.concatenate(
        [W1] + [W2[d] for d in range(3)] + [WFm[d] for d in range(3)]
        + [onesel, selback, ident32, ones11], axis=1)

    g1 = np.asarray(inputs['bn1_g'], np.float32)
    b1 = np.asarray(inputs['bn1_b'], np.float32)
    g2 = np.asarray(inputs['bn2_g'], np.float32)
    b2 = np.asarray(inputs['bn2_b'], np.float32)
    fb = np.asarray(inputs['convf_b'], np.float32)
    CF = np.zeros((128, 8), np.float32)
    CF[:32, 0] = g1
    CF[:32, 1] = b1
    CF[:32, 2] = g2
    CF[:32, 3] = b2
    CF[:16, 4] = fb[0]
    CF[16:32, 4] = fb[1]
    CF[:, 6] = BN_EPS   # col 5 stays zero (zero-bias AP)

    y = np.asarray(inputs['y'], np.float32)
    ybm_cores, ylhsT_cores = [], []
    for c in range(NCORE):
        ys = y[c * BS:(c + 1) * BS]
        ybm = np.concatenate([ys[:, 0], ys[:, 1]], 0)    # [32, Nv]
        ybm_cores.append(np.ascontiguousarray(ybm))
        sT = ybm.T                                       # [Nv, 32]
        bot = np.concatenate([sT[:, 16:], -sT[:, :16]], 1)
        comboH = np.concatenate([sT, bot], 0)            # [2Nv, 32]
        ylhsT_cores.append(
            comboH.reshape(8, 128, 32).transpose(1, 0, 2).copy())  # [128, 8, 32]

    return dict(AB=AB, ATD=ATD, AAHD=AAHD, minv_stacks=minv_stacks,
                iter_minv_idx=iter_minv_idx, rhos=rhos, epss=epss,
                WTS=WTS, CF=CF, ybm_cores=ybm_cores, ylhsT_cores=ylhsT_cores)


# WTS column offsets
W1_C = 0
W2_C = 128
WF_C = 128 + 384
OSEL_C = WF_C + 24
SELB_C = OSEL_C + 32
ID32_C = SELB_C + 128
ONE1_C = ID32_C + 32
WTS_W = ONE1_C + 1


# ---------------------------------------------------------------- program
def _build_program(prep):
    import concourse.bacc as bacc
    import concourse.tile as tile
    import concourse.mybir as mybir

    dt = mybir.dt
    f32, f32r = dt.float32, dt.float32r
    AX = mybir.AxisListType
    OP = mybir.AluOpType
    AF = mybir.ActivationFunctionType

    nu = len(prep['minv_stacks'])
    rhos, epss = prep['rhos'], prep['epss']
    cnt = float(B * Nt)

    nc = bacc.Bacc("TRN2", target_bir_lowering=False, debug=False,
                   num_devices=NCORE)

    AB_d = nc.dram_tensor("AB", [128, 8, 2048], f32r, kind="ExternalInput")
    AT_d = nc.dram_tensor("ATD", [32, 128, 512], f32r, kind="ExternalInput")
    AAH_d = nc.dram_tensor("AAHD", [128, 8, 512], f32r, kind="ExternalInput")
    MINV_d = nc.dram_tensor("MINVS", [nu, 128, 8, 512], f32r, kind="ExternalInput")
    WTS_d = nc.dram_tensor("WTS", [128, WTS_W], f32r, kind="ExternalInput")
    CF_d = nc.dram_tensor("CF", [128, 8], f32, kind="ExternalInput")
    Y_d = nc.dram_tensor("YBM", [32, 512], f32r, kind="ExternalInput")
    YL_d = nc.dram_tensor("YL", [128, 8, 32], f32r, kind="ExternalInput")
    XO_d = nc.dram_tensor("XOUT", [32, 2048], f32r, kind="ExternalOutput")
    DBG_d = nc.dram_tensor("DBG", [32, 2048], f32r, kind="ExternalOutput")
    DBG2_d = nc.dram_tensor("DBG2", [128, 4096], f32r, kind="ExternalOutput")
    DBG4_d = nc.dram_tensor("DBG4", [32, 3, 2048], f32r, kind="ExternalOutput")
    DBG3_d = nc.dram_tensor("DBG3", [32, 4096], f32r, kind="ExternalOutput")

    with tile.TileContext(nc) as tc:
        with (
            tc.tile_pool(name="cst", bufs=1) as cst,
            tc.tile_pool(name="atp", bufs=4) as atp,
            tc.tile_pool(name="st", bufs=1) as stp,
            tc.tile_pool(name="cmb", bufs=1) as cmb,
            tc.tile_pool(name="act", bufs=5) as actp,
            tc.tile_pool(name="xin", bufs=1) as xinp,
            tc.tile_pool(name="sc", bufs=1) as scp,
            tc.tile_pool(name="psA", bufs=3, space="PSUM") as psA,
            tc.tile_pool(name="psB", bufs=2, space="PSUM") as psB,
            tc.tile_pool(name="psC", bufs=1, space="PSUM") as psC,
            tc.tile_pool(name="psD", bufs=2, space="PSUM") as psD,
            tc.tile_pool(name="ddr", bufs=2, space="DRAM") as ddr,
        ):
            # ---- constants into SBUF ----
            ab = cst.tile([128, 8, 2048], f32r, tag="ab")
            aah = cst.tile([128, 8, 512], f32r, tag="aah")
            minv = cst.tile([128, 8, 512], f32r, tag="minv")
            wts = cst.tile([128, WTS_W], f32r, tag="wts")
            cf = cst.tile([128, 8], f32, tag="cf")
            yl = cst.tile([128, 8, 32], f32r, tag="yl")
            nc.sync.dma_start(ab[:], AB_d[:])
            nc.sync.dma_start(aah[:], AAH_d[:])
            nc.sync.dma_start(wts[:], WTS_d[:])
            nc.sync.dma_start(cf[:], CF_d[:])
            nc.sync.dma_start(yl[:], YL_d[:])
            if nu == 1:
                nc.sync.dma_start(minv[:], MINV_d[0])

            W1 = wts[:, W1_C:W1_C + 128]
            W2 = [wts[:, W2_C + 128 * d: W2_C + 128 * (d + 1)] for d in range(3)]
            WF = [wts[:, WF_C + 8 * d: WF_C + 8 * (d + 1)] for d in range(3)]
            OSEL = wts[:, OSEL_C:OSEL_C + 32]
            SELB = wts[0:32, SELB_C:SELB_C + 128]
            ID32 = wts[0:32, ID32_C:ID32_C + 32]
            ONE1 = wts[0:1, ONE1_C:ONE1_C + 1]
            g32 = [cf[0:32, 0:1], cf[0:32, 2:3]]
            b32 = [cf[0:32, 1:2], cf[0:32, 3:4]]
            fb32 = cf[0:32, 4:5]
            zb128 = cf[:, 5:6]
            zb32 = cf[0:32, 5:6]
            zb1 = cf[0:1, 5:6]
            epsb = cf[0:32, 6:7]

            # ---- state ----
            X2 = stp.tile([32, 2, 2048], f32r, tag="X2")     # 0: x, 1: rc1
            x_t = X2[:, 0, :]
            rc1_t = X2[:, 1, :]
            S = stp.tile([32, 8, 512], f32r, tag="S")
            z_t, u_t, y_t = S[:, 0, :], S[:, 1, :], S[:, 2, :]
            atx_t, tmv_t = S[:, 3, :], S[:, 4, :]
            v_t, dif_t, arc_t = S[:, 5, :], S[:, 6, :], S[:, 7, :]
            sq_t = stp.tile([32, 512], f32r, tag="sq")
            s32f = stp.tile([32, 1], f32, tag="s32f")
            nc.sync.dma_start(y_t[:], Y_d[:])
            nc.vector.memset(u_t[:].bitcast(f32), 0.0)

            zmuT = cmb.tile([128, 12, 32], f32r, tag="zmuT")
            atxT = cmb.tile([128, 12, 32], f32r, tag="atxT")
            tmvT = cmb.tile([128, 12, 32], f32r, tag="tmvT")
            rcT = cmb.tile([128, 32, 32], f32r, tag="rcT")
            smal = cmb.tile([32, 8], f32r, tag="smal")
            gb_t = smal[:, 1:3]
            mean_t, var_t = smal[:, 3:4], smal[:, 4:5]
            ssn_t, m2_t = smal[:, 5:6], smal[:, 6:7]
            row1 = cmb.tile([1, 96], f32, tag="row1")       # factor chain
            n2_t = row1[:, 0:16]
            nrm_t = row1[:, 16:32]
            fac_t = row1[:, 32:48]
            facd_t = row1[:, 48:64]  # unused half; facd uses 32 cols below
            gbb = cmb.tile([128, 2], f32, tag="gbb")
            stat = cmb.tile([128, 4, 16], f32, tag="stat")  # sums/sumsq per tile
            stat2 = cmb.tile([128, 2], f32, tag="stat2")
            stat2r = cmb.tile([128, 2], f32r, tag="stat2r")

            def combo_build(dst, src_bm, nchunk, kinds):
                """dst [128, 12|32, 32]; src_bm [32, nchunk*128]; kinds subset
                of {'H','N'}. chunks: 0..nc-1 top; nc..2nc-1 H-bot or N-bot at
                fixed offsets: H at nchunk, N at 2*nchunk (for 12-layout: 4/8)."""
                for c in range(nchunk):
                    pT = psD.tile([128, 32], f32r, tag="tp")
                    nc.tensor.transpose(pT[:], src_bm[:, 128 * c:128 * (c + 1)],
                                        ID32)
                    nc.vector.tensor_copy(dst[:, c, :], pT[:])
                    if 'H' in kinds:
                        o = nchunk
                        nc.vector.tensor_copy(dst[:, o + c, 0:16],
                                              pT[:, 16:32])
                        nc.vector.tensor_scalar_mul(dst[:, o + c, 16:32],
                                                    pT[:, 0:16], -1.0)
                    if 'N' in kinds:
                        o = nchunk if nchunk == 16 else 2 * nchunk
                        nc.vector.tensor_scalar_mul(dst[:, o + c, 0:16],
                                                    pT[:, 16:32], -1.0)
                        nc.vector.tensor_copy(dst[:, o + c, 16:32],
                                              pT[:, 0:16])

            def mm_chunks(psum, lhsT_tile, rhs, kmap, n0, nn):
                """psum [32, nn] += sum_k lhsT[:,kmap[k],:].T @ rhs[:,k,n0:n0+nn]"""
                nk = len(kmap)
                for ki, kc in enumerate(kmap):
                    nc.tensor.matmul(psum[:], lhsT_tile[:, kc, :],
                                     rhs[:, ki, n0:n0 + nn],
                                     start=(ki == 0), stop=(ki == nk - 1))

            KH = [0, 1, 2, 3, 4, 5, 6, 7]          # comboH chunks in 12-layout
            KN = [0, 1, 2, 3, 8, 9, 10, 11]        # comboN chunks in 12-layout

            # ---- x0 = A^H y ----
            for nt in range(4):
                p = psA.tile([32, 512], f32, tag="mm")
                for k in range(8):
                    nc.tensor.matmul(p[:], yl[:, k, :], ab[:, k, 512 * nt:512 * (nt + 1)],
                                     start=(k == 0), stop=(k == 7))
                nc.vector.tensor_copy(x_t[:, 512 * nt:512 * (nt + 1)], p[:])

            # ================= iterations =================
            for it in range(ITERS):
                rho = float(rhos[it])
                eps = float(epss[it])
                c1 = 1.0 / (rho + 1e-8)
                if nu > 1:
                    nc.sync.dma_start(minv[:], MINV_d[prep['iter_minv_idx'][it]])

                # ---------- CNN ----------
                act1 = []
                for bp in range(4):
                    xin = xinp.tile([32, 2048], f32r, tag="xin")
                    nc.vector.memset(xin[:, 0:1].bitcast(f32), 0.0)
                    nc.vector.memset(xin[:, 2047:2048].bitcast(f32), 0.0)
                    for dl in range(3):
                        lo, hi = max(0, 1 - dl), min(2048, 2048 + 1 - dl)
                        for ci in range(2):
                            src = x_t[ci * 16 + bp * 4: ci * 16 + bp * 4 + 4,
                                      lo + dl - 1: hi + dl - 1]
                            nc.sync.dma_start(
                                xin[dl * 8 + ci * 4: dl * 8 + ci * 4 + 4, lo:hi], src)
                    a1 = actp.tile([128, 2050], f32r, tag="act")
                    nc.vector.memset(a1[:, 0:1].bitcast(f32), 0.0)
                    nc.vector.memset(a1[:, 2049:2050].bitcast(f32), 0.0)
                    for lt in range(4):
                        p = psB.tile([128, 512], f32, tag="big")
                        nc.tensor.matmul(p[:], W1[0:24, :],
                                         xin[0:24, 512 * lt:512 * (lt + 1)],
                                         start=True, stop=True)
                        idx = bp * 4 + lt
                        nc.vector.tensor_copy(
                            a1[:, 1 + 512 * lt:1 + 512 * (lt + 1)], p[:])
                        sj = scp.tile([128, 512], f32, tag="sqj")
                        nc.scalar.activation(
                            sj[:], p[:], AF.Square, bias=zb128[:],
                            accum_out=stat[:, idx // 4, 4 + idx % 4:5 + idx % 4])
                    nc.vector.tensor_reduce(stat[:, bp, 8:9],
                                            a1[:, 1:2049], AX.X, OP.add)
                    act1.append(a1)

                def bn_apply(layer, acts):
                    # stat[:, t, 0:4] sums, [:, t, 4:8] sumsq for 4 l-tiles
                    with nc.allow_low_precision(reason="f32r rounding of fp32 sums"):
                        nc.vector.tensor_reduce(stat2[:, 0:1], stat[:, :, 8:9],
                                                AX.XY, OP.add)
                        nc.vector.tensor_reduce(stat2[:, 1:2], stat[:, :, 4:8],
                                                AX.XY, OP.add)
                    ci_ = ddr.tile([128, 2], f32, tag="cc")
                    co_ = ddr.tile([128, 2], f32, tag="cc")
                    nc.sync.dma_start(ci_[:], stat2[:])
                    nc.gpsimd.collective_compute(
                        "AllReduce", OP.add, replica_groups=[list(range(NCORE))],
                        ins=[ci_.opt()], outs=[co_.opt()])
                    nc.sync.dma_start(stat2[:], co_[:])
                    nc.vector.tensor_copy(stat2r[:], stat2[:])
                    p = psC.tile([32, 2], f32, tag="sm")
                    nc.tensor.matmul(p[:], OSEL, stat2r[:],
                                     start=True, stop=True)
                    with nc.allow_low_precision(reason="bn scalar math in f32r"):
                        nc.vector.tensor_scalar_mul(mean_t[:], p[:, 0:1], 1.0 / cnt)
                        nc.vector.tensor_scalar_mul(ssn_t[:], p[:, 1:2], 1.0 / cnt)
                        nc.vector.tensor_mul(m2_t[:], mean_t[:], mean_t[:])
                        nc.vector.tensor_sub(var_t[:], ssn_t[:], m2_t[:])
                        nc.scalar.activation(var_t[:], var_t[:], AF.Sqrt, bias=epsb[:])
                        nc.vector.reciprocal(var_t[:], var_t[:])
                        nc.vector.tensor_mul(gb_t[:, 0:1], g32[layer][:], var_t[:])
                        nc.vector.tensor_mul(m2_t[:], mean_t[:], gb_t[:, 0:1])
                        nc.vector.tensor_sub(gb_t[:, 1:2], b32[layer][:], m2_t[:])
                    p2 = psC.tile([128, 2], f32, tag="sm")
                    nc.tensor.matmul(p2[:], SELB, gb_t[:],
                                     start=True, stop=True)
                    nc.vector.tensor_copy(gbb[:], p2[:])
                    for a in acts:
                        nc.scalar.activation(a[:, 1:2049], a[:, 1:2049], AF.Relu,
                                             bias=gbb[:, 1:2], scale=gbb[:, 0:1])

                bn_apply(0, act1)
                if it == 0:
                    nc.sync.dma_start(DBG2_d[:, 0:2048], act1[0][:, 1:2049])

                # conv2
                act2 = []
                for bp in range(4):
                    a2 = actp.tile([128, 2050], f32r, tag="act")
                    nc.vector.memset(a2[:, 0:1].bitcast(f32), 0.0)
                    nc.vector.memset(a2[:, 2049:2050].bitcast(f32), 0.0)
                    for lt in range(4):
                        p = psB.tile([128, 512], f32, tag="big")
                        for dl in range(3):
                            nc.tensor.matmul(
                                p[:], W2[dl],
                                act1[bp][:, dl + 512 * lt: dl + 512 * (lt + 1)],
                                start=(dl == 0), stop=(dl == 2))
                        idx = bp * 4 + lt
                        nc.vector.tensor_copy(
                            a2[:, 1 + 512 * lt:1 + 512 * (lt + 1)], p[:])
                        sj = scp.tile([128, 512], f32, tag="sqj")
                        nc.scalar.activation(
                            sj[:], p[:], AF.Square, bias=zb128[:],
                            accum_out=stat[:, idx // 4, 4 + idx % 4:5 + idx % 4])
                    nc.vector.tensor_reduce(stat[:, bp, 8:9],
                                            a2[:, 1:2049], AX.X, OP.add)
                    act2.append(a2)

                bn_apply(1, act2)
                if it == 0:
                    nc.sync.dma_start(DBG2_d[:, 2048:4096], act2[0][:, 1:2049])

                # convf + residual: rc1 = (x + convf + fb) * c1
                for bp in range(4):
                    for lt in range(4):
                        p = psC.tile([8, 512], f32, tag="sm")
                        for dl in range(3):
                            nc.tensor.matmul(
                                p[:], WF[dl],
                                act2[bp][:, dl + 512 * lt: dl + 512 * (lt + 1)],
                                start=(dl == 0), stop=(dl == 2))
                        s8 = scp.tile([8, 512], f32r, tag="s8")
                        nc.vector.tensor_copy(s8[:], p[:])
                        for cfi in range(2):
                            nc.sync.dma_start(
                                rc1_t[cfi * 16 + bp * 4: cfi * 16 + bp * 4 + 4,
                                      512 * lt:512 * (lt + 1)],
                                s8[cfi * 4:cfi * 4 + 4, :])
                nc.vector.tensor_add(rc1_t[:], rc1_t[:], x_t[:])
                nc.scalar.activation(rc1_t[:], rc1_t[:], AF.Identity,
                                     bias=fb32[:], scale=c1)
                if it == 1:
                    nc.sync.dma_start(DBG_d[:], rc1_t[:])


                # arc = A * rc1 (comboN(rc1) against AT chunks, streamed)
                combo_build(rcT, rc1_t, 16, {'N'})
                parc = psA.tile([32, 512], f32, tag="mm")
                for k in range(32):
                    atc = atp.tile([128, 512], f32r, tag="atc")
                    nc.sync.dma_start(atc[:], AT_d[k])
                    kc = k if k < 16 else k  # rcT: top 0-15, N-bot 16-31
                    nc.tensor.matmul(parc[:], rcT[:, kc, :],
                                     atc[:], start=(k == 0), stop=(k == 31))
                nc.vector.tensor_copy(arc_t[:], parc[:])

                # ---------- ADMM ----------
                for s in range(ADMM):
                    final = (s == ADMM - 1)
                    # zmu
                    zsrc = y_t if s == 0 else z_t
                    nc.vector.tensor_sub(sq_t[:], zsrc[:], u_t[:])  # sq_t = zmu
                    combo_build(zmuT, sq_t, 4, {'N', 'H'} if final else {'N'})
                    # aahz -> Atx = arc + rho*c1*aahz
                    p = psA.tile([32, 512], f32, tag="mm")
                    mm_chunks(p, zmuT, aah, KN, 0, 512)
                    nc.vector.scalar_tensor_tensor(atx_t[:], p[:], rho * c1,
                                                   arc_t[:], OP.mult, OP.add)
                    if it == 0:
                        nc.sync.dma_start(DBG3_d[:, 1024 * s:1024 * s + 512], atx_t[:])
                    combo_build(atxT, atx_t, 4, {'N'})
                    # tmv = Minv * Atx
                    p = psA.tile([32, 512], f32, tag="mm")
                    mm_chunks(p, atxT, minv, KN, 0, 512)
                    nc.vector.tensor_copy(tmv_t[:], p[:])
                    combo_build(tmvT, tmv_t, 4, {'N', 'H'} if final else {'N'})
                    # Ax = Atx - AAH*tmv
                    p = psA.tile([32, 512], f32, tag="mm")
                    mm_chunks(p, tmvT, aah, KN, 0, 512)
                    nc.vector.tensor_sub(v_t[:], atx_t[:], p[:])   # v_t = Ax
                    if final:
                        for nt in range(4):
                            p = psA.tile([32, 512], f32, tag="mm")
                            mm_chunks(p, zmuT, ab, KH, 512 * nt, 512)
                            nc.vector.scalar_tensor_tensor(
                                x_t[:, 512 * nt:512 * (nt + 1)], p[:], rho * c1,
                                rc1_t[:, 512 * nt:512 * (nt + 1)], OP.mult, OP.add)
                        for nt in range(4):
                            p = psA.tile([32, 512], f32, tag="mm")
                            mm_chunks(p, tmvT, ab, KH, 512 * nt, 512)
                            nc.vector.tensor_sub(x_t[:, 512 * nt:512 * (nt + 1)],
                                                 x_t[:, 512 * nt:512 * (nt + 1)], p[:])
                    # projection
                    nc.vector.tensor_add(v_t[:], v_t[:], u_t[:])   # v = Ax + u
                    nc.vector.tensor_sub(dif_t[:], v_t[:], y_t[:])
                    nc.vector.scalar_tensor_tensor(sq_t[:], dif_t[:], 1.0,
                                                   dif_t[:], OP.mult, OP.mult,
                                                   accum_out=s32f[:])
                    pt = psC.tile([1, 32], f32, tag="sm")
                    nc.tensor.matmul(pt[:], s32f[:], ID32.bitcast(f32),
                                     is_transpose=True)
                    nc.vector.tensor_copy(row1[:, 64:96], pt[:])
                    nc.vector.tensor_add(n2_t[:], row1[:, 64:80], row1[:, 80:96])
                    nc.scalar.activation(nrm_t[:], n2_t[:], AF.Sqrt, bias=zb1[:])
                    nc.vector.tensor_scalar_add(nrm_t[:], nrm_t[:], 1e-12)
                    nc.vector.reciprocal(nrm_t[:], nrm_t[:])
                    nc.vector.tensor_scalar_mul(fac_t[:], nrm_t[:], eps)
                    nc.vector.tensor_scalar_min(fac_t[:], fac_t[:], 1.0)
                    fr = psC.tile([32, 1], f32, tag="sm")
                    nc.vector.tensor_copy(row1[:, 48:64], fac_t[:])
                    nc.tensor.matmul(fr[:], row1[:, 32:64], ONE1.bitcast(f32),
                                     is_transpose=True)
                    nc.vector.scalar_tensor_tensor(z_t[:], dif_t[:], fr[:],
                                                   y_t[:], OP.mult, OP.add)
                    nc.vector.tensor_sub(u_t[:], v_t[:], z_t[:])
                    if it == 0:
                        nc.sync.dma_start(DBG3_d[:, 1024 * s + 512:1024 * (s + 1)], z_t[:])

                if 1 <= it <= 3:
                    nc.sync.dma_start(DBG4_d[:, it - 1, :], x_t[:])

            nc.sync.dma_start(XO_d[:], x_t[:])

    nc.compile()
    return nc


_CACHE = {}


def _enable_trace_shim():
    import sys, types
    try:
        import trn_agent_boot.trn_boot as _tb
        import concourse.bass_utils as _bu
        _bu.upload_artifacts = lambda tmpdir: "local://" + str(tmpdir)
        hookmod = types.ModuleType('antenv.axon_hooks')
        hook = _tb._ntff_profile_via_ctypes('/opt/axon/libaxon_pjrt.so')
        hookmod.get_axon_ntff_profile_hook = lambda: hook
        import antenv as _antenv
        sys.modules['antenv.axon_hooks'] = hookmod
        _antenv.axon_hooks = hookmod
        return True
    except Exception:
        return False


def kernel(**inputs) -> np.ndarray:
    import os
    from concourse.bass_utils import run_bass_kernel_spmd
    trace = bool(os.environ.get("KERNEL_TRACE"))
    if trace:
        trace = _enable_trace_shim()

    prep = _host_prep(inputs)
    key = "prog"
    nc = _build_program(prep)

    minvs = np.stack(prep['minv_stacks'], 0)
    in_maps = []
    for c in range(NCORE):
        in_maps.append({
            "AB": prep['AB'], "ATD": prep['ATD'], "AAHD": prep['AAHD'],
            "MINVS": minvs, "WTS": prep['WTS'], "CF": prep['CF'],
            "YBM": np.ascontiguousarray(prep['ybm_cores'][c][:, :512]),
            "YL": prep['ylhsT_cores'][c],
        })
    res = run_bass_kernel_spmd(nc, in_maps, list(range(NCORE)), trace=trace)
    out = np.zeros((B, 2, Nt), np.float32)
    for c in range(NCORE):
        xc = res.results[c]["XOUT"]
        out[c * BS:(c + 1) * BS, 0] = xc[:16]
        out[c * BS:(c + 1) * BS, 1] = xc[16:]
    kernel._last_results = res
    return out



# revision 9
# speedup vs baseline: 1.7682x; 1.7682x over previous
"""DBPNet Trainium2 kernel: 8-core data-parallel Bass/Tile implementation.

v2 scheme:
  - ADMM algebra: with M = (AAH + rho I)^-1,  AAH@M = I - rho*M, so
      tmv = marc + rho*c1*zmu - rho^2*c1*mz      (mz = M@zmu, marc = M@arc)
      Ax  = rho*tmv   (no AAH matmuls on device at all)
      x   = rc1 + A^H(rho^2*c1*mz - marc)
    -> one 8-matmul group (mz) per ADMM step.
  - proj state: u' = (1-fac)*diff; zmu'(within iter) = y + (2fac-1)*diff;
    zmu0(next iter) = y + (fac-1)*diff.
  - combo transposes via wide signed identities [I|J]: one PE op -> top+bottom.
  - CNN in bf16 single-tile activations [128, 4bp, 2050], block-diag weights.
  - arc path (AT stream + rcT) in bf16; everything else f32r.
  - BN stats pre-reduced to [32,2] and packed [2,32] before the collective
    (2/16-descriptor DMAs); AllGather + local reduce.
"""
import numpy as np

B, Nv, Nt, F = 128, 512, 2048, 32
NCORE, BS = 8, 16
ITERS, ADMM = 5, 3
BN_EPS = 1e-5
USE_AG = True


# ---------------------------------------------------------------- host prep
def _host_prep(inputs):
    import ml_dtypes
    A = np.ascontiguousarray(np.asarray(inputs['A'], np.float32))
    Ar, Ai = A[0], A[1]
    Ac = Ar.astype(np.float64) + 1j * Ai.astype(np.float64)
    AAH = Ac @ Ac.conj().T

    rhos = np.exp(np.asarray(inputs['log_rho'], np.float32)).astype(np.float32)
    epss = np.exp(np.asarray(inputs['log_eps'], np.float32)).astype(np.float32)

    minv_stacks, rho_to_idx, iter_minv_idx = [], {}, []
    for r in rhos:
        key = float(r)
        if key not in rho_to_idx:
            M = np.linalg.inv(AAH + key * np.eye(Nv))
            Mr = M.real.astype(np.float32)
            Mi = M.imag.astype(np.float32)
            minv_stacks.append(
                np.concatenate([Mr.T, Mi.T], 0).reshape(8, 128, 512)
                .transpose(1, 0, 2).copy())            # [128, 8, 512]
            rho_to_idx[key] = len(minv_stacks) - 1
        iter_minv_idx.append(rho_to_idx[float(r)])
    nu = len(minv_stacks)

    A1 = np.concatenate([Ar, Ai], 0)                    # [1024, 2048]
    AB = A1.reshape(8, 128, 2048).transpose(1, 0, 2).copy()   # [128, 8, 2048]
    # AT stream half-groups (bf16): per lt: tops01, tops23, bots01, bots23.
    AT1 = np.concatenate([Ar.T, Ai.T], 0)               # [4096, 512]
    ATtiles = AT1.reshape(32, 128, 512)
    ATG = np.zeros((16, 128, 2, 512), np.float32)
    for lt in range(4):
        for h in range(2):
            for j in range(2):
                ATG[4 * lt + h, :, j, :] = ATtiles[lt * 4 + 2 * h + j]
                ATG[4 * lt + 2 + h, :, j, :] = ATtiles[16 + lt * 4 + 2 * h + j]
    ATGb = ATG.astype(ml_dtypes.bfloat16)

    # bf16 CNN weights pack [128, 896]
    w1 = np.asarray(inputs['conv1_w'], np.float32)
    w2 = np.asarray(inputs['conv2_w'], np.float32)
    wf = np.asarray(inputs['convf_w'], np.float32)
    W1 = np.zeros((128, 128), np.float32)
    for dl in range(3):
        for ci in range(2):
            for q in range(4):
                W1[dl * 8 + ci * 4 + q, np.arange(F) * 4 + q] = w1[:, ci, dl]
    W2 = np.zeros((3, 128, 128), np.float32)
    for dl in range(3):
        for ci in range(F):
            for q in range(4):
                W2[dl, ci * 4 + q, np.arange(F) * 4 + q] = w2[:, ci, dl]
    WFb = np.zeros((4, 3, 128, 32), np.float32)
    for bp in range(4):
        for dl in range(3):
            for ci in range(F):
                for q in range(4):
                    for cf in range(2):
                        WFb[bp, dl, ci * 4 + q, cf * 16 + bp * 4 + q] = wf[cf, ci, dl]
    WTB = np.concatenate(
        [W1] + [W2[d] for d in range(3)]
        + [WFb[bp, dl] for bp in range(4) for dl in range(3)],
        axis=1).astype(ml_dtypes.bfloat16)              # [128, 896]

    # f32 selector/identity pack
    OSEL = np.zeros((128, 32), np.float32)
    SELB = np.zeros((128, 128), np.float32)   # rows 0-31 used
    for co in range(32):
        for q in range(4):
            OSEL[co * 4 + q, co] = 1.0
            SELB[co, co * 4 + q] = 1.0
    I16 = np.eye(16, dtype=np.float32)
    JN = np.zeros((32, 32), np.float32)
    JN[16:, :16] = -I16
    JN[:16, 16:] = I16
    JH = -JN
    ID32 = np.eye(32, dtype=np.float32)
    IDJN = np.zeros((128, 64), np.float32)
    IDJN[:32, :32] = ID32
    IDJN[:32, 32:] = JN
    IDJH = np.zeros((128, 64), np.float32)
    IDJH[:32, :32] = ID32
    IDJH[:32, 32:] = JH
    P32 = np.zeros((128, 32), np.float32)
    for m in range(32):
        P32[m, m] = 1.0
        P32[(m + 16) % 32, m] = 1.0
    ID32p = np.zeros((128, 32), np.float32)
    ID32p[:32, :32] = ID32
    WTS = np.concatenate([OSEL, SELB, IDJN, IDJH, P32, ID32p], axis=1)

    g1 = np.asarray(inputs['bn1_g'], np.float32)
    b1 = np.asarray(inputs['bn1_b'], np.float32)
    g2 = np.asarray(inputs['bn2_g'], np.float32)
    b2 = np.asarray(inputs['bn2_b'], np.float32)
    fb = np.asarray(inputs['convf_b'], np.float32)
    CF = np.zeros((128, 8), np.float32)
    CF[:32, 0] = g1
    CF[:32, 1] = b1
    CF[:32, 2] = g2
    CF[:32, 3] = b2
    CF[:, 6] = BN_EPS
    FBC = np.zeros((128, nu), np.float32)
    for key, idx in rho_to_idx.items():
        c1 = 1.0 / (key + 1e-8)
        FBC[:16, idx] = fb[0] * c1
        FBC[16:32, idx] = fb[1] * c1

    y = np.asarray(inputs['y'], np.float32)
    ybm_cores, ylh_cores, yln_cores = [], [], []
    for c in range(NCORE):
        ys = y[c * BS:(c + 1) * BS]
        ybm = np.concatenate([ys[:, 0], ys[:, 1]], 0)    # [32, Nv]
        ybm_cores.append(np.ascontiguousarray(ybm))
        sT = ybm.T                                       # [Nv, 32]
        botH = np.concatenate([sT[:, 16:], -sT[:, :16]], 1)
        botN = np.concatenate([-sT[:, 16:], sT[:, :16]], 1)
        comboH = np.concatenate([sT, botH], 0)           # [2Nv, 32]
        comboN = np.concatenate([sT, botN], 0)
        ylh_cores.append(comboH.reshape(8, 128, 32).transpose(1, 0, 2).copy())
        yln_cores.append(comboN.reshape(8, 128, 32).transpose(1, 0, 2).copy())

    return dict(AB=AB, ATG=ATGb, minv_stacks=minv_stacks,
                iter_minv_idx=iter_minv_idx, rhos=rhos, epss=epss,
                WTS=WTS, WTB=WTB, CF=CF, FBC=FBC, ybm_cores=ybm_cores,
                ylh_cores=ylh_cores, yln_cores=yln_cores)


# WTS column offsets
OSEL_C = 0
SELB_C = 32
IDJN_C = SELB_C + 128
IDJH_C = IDJN_C + 64
P32_C = IDJH_C + 64
ID32_C = P32_C + 32
WTS_W = ID32_C + 32
# WTB column offsets
W1_C = 0
W2_C = 128
WF_C = W2_C + 384
WTB_W = WF_C + 384


# ---------------------------------------------------------------- program
def _build_program(prep):
    import concourse.bacc as bacc
    import concourse.tile as tile
    import concourse.mybir as mybir

    dt = mybir.dt
    f32, f32r, bf16 = dt.float32, dt.float32r, dt.bfloat16
    AX = mybir.AxisListType
    OP = mybir.AluOpType
    AF = mybir.ActivationFunctionType

    nu = len(prep['minv_stacks'])
    rhos, epss = prep['rhos'], prep['epss']
    cnt = float(B * Nt)

    nc = bacc.Bacc("TRN2", target_bir_lowering=False, debug=False,
                   num_devices=NCORE)

    AB_d = nc.dram_tensor("AB", [128, 8, 2048], f32r, kind="ExternalInput")
    ATG_d = nc.dram_tensor("ATG", [16, 128, 2, 512], bf16, kind="ExternalInput")
    MINV_d = nc.dram_tensor("MINVS", [nu, 128, 8, 512], f32r, kind="ExternalInput")
    WTS_d = nc.dram_tensor("WTS", [128, WTS_W], f32r, kind="ExternalInput")
    WTB_d = nc.dram_tensor("WTB", [128, WTB_W], bf16, kind="ExternalInput")
    CF_d = nc.dram_tensor("CF", [128, 8], f32, kind="ExternalInput")
    FBC_d = nc.dram_tensor("FBC", [128, nu], f32, kind="ExternalInput")
    Y_d = nc.dram_tensor("YBM", [32, 512], f32r, kind="ExternalInput")
    YLH_d = nc.dram_tensor("YLH", [128, 8, 32], f32r, kind="ExternalInput")
    YLN_d = nc.dram_tensor("YLN", [128, 8, 32], f32r, kind="ExternalInput")
    XO_d = nc.dram_tensor("XOUT", [32, 2048], f32r, kind="ExternalOutput")

    with tile.TileContext(nc) as tc:
        with (
            tc.tile_pool(name="cst", bufs=1) as cst,
            tc.tile_pool(name="atp", bufs=8) as atp,
            tc.tile_pool(name="st", bufs=1) as stp,
            tc.tile_pool(name="cnn", bufs=1) as cnnp,
            tc.tile_pool(name="xin", bufs=4) as xinp,
            tc.tile_pool(name="psbig", bufs=2, space="PSUM") as psbig,
            tc.tile_pool(name="psgen", bufs=2, space="PSUM") as psgen,
            tc.tile_pool(name="psarc", bufs=1, space="PSUM") as psarc,
            tc.tile_pool(name="pssm", bufs=1, space="PSUM") as pssm,
            tc.tile_pool(name="ddr", bufs=2, space="DRAM") as ddr,
        ):
            # ---- constants ----
            ab = cst.tile([128, 8, 2048], f32r, tag="ab")
            minv = cst.tile([128, 8, 512], f32r, tag="minv")
            wts = cst.tile([128, WTS_W], f32r, tag="wts")
            wtb = cst.tile([128, WTB_W], bf16, tag="wtb")
            cf = cst.tile([128, 8], f32, tag="cf")
            fbc = cst.tile([128, nu], f32, tag="fbc")
            ylh = cst.tile([128, 8, 32], f32r, tag="ylh")
            yln = cst.tile([128, 8, 32], f32r, tag="yln")
            nc.sync.dma_start(ab[:], AB_d[:])
            nc.sync.dma_start(wts[:], WTS_d[:])
            nc.sync.dma_start(wtb[:], WTB_d[:])
            nc.sync.dma_start(cf[:], CF_d[:])
            nc.sync.dma_start(fbc[:], FBC_d[:])
            nc.sync.dma_start(ylh[:], YLH_d[:])
            nc.sync.dma_start(yln[:], YLN_d[:])
            if nu == 1:
                nc.sync.dma_start(minv[:], MINV_d[0])

            OSEL = wts[:, OSEL_C:OSEL_C + 32]
            SELB = wts[0:32, SELB_C:SELB_C + 128]
            IDJN = wts[0:32, IDJN_C:IDJN_C + 64]
            IDJH = wts[0:32, IDJH_C:IDJH_C + 64]
            P32 = wts[0:32, P32_C:P32_C + 32]
            ID32 = wts[0:32, ID32_C:ID32_C + 32]
            W1 = wtb[0:24, W1_C:W1_C + 128]
            W2 = [wtb[:, W2_C + 128 * d: W2_C + 128 * (d + 1)] for d in range(3)]
            WF = [[wtb[:, WF_C + (bp * 3 + d) * 32: WF_C + (bp * 3 + d) * 32 + 32]
                   for d in range(3)] for bp in range(4)]
            g32 = [cf[0:32, 0:1], cf[0:32, 2:3]]
            b32 = [cf[0:32, 1:2], cf[0:32, 3:4]]
            zb128 = cf[:, 5:6]
            zb32 = cf[0:32, 5:6]
            epsb = cf[0:32, 6:7]

            # ---- state ----
            X = stp.tile([32, 2048], f32r, tag="X")
            RC = stp.tile([32, 2048], f32r, tag="RC")
            XB = stp.tile([32, 2050], bf16, tag="XB")
            S = stp.tile([32, 8, 512], f32r, tag="S")
            y_t, u_t, zmu_t, marc_t = S[:, 0, :], S[:, 1, :], S[:, 2, :], S[:, 3, :]
            q3_t, t1_t, dif_t, sq_t = S[:, 4, :], S[:, 5, :], S[:, 6, :], S[:, 7, :]
            arcS = stp.tile([32, 512], f32r, tag="arcS")
            mzS = stp.tile([32, 512], f32r, tag="mzS")
            s32f = stp.tile([32, 1], f32, tag="s32f")
            facs = stp.tile([32, 4], f32, tag="facs")
            smal = stp.tile([32, 8], f32r, tag="smal")
            gbt = stp.tile([32, 2], f32r, tag="gbt")
            gbb = stp.tile([128, 2], f32, tag="gbb")
            zmuT = stp.tile([128, 8, 32], f32r, tag="zmuT")
            wT = stp.tile([128, 8, 32], f32r, tag="wT")
            rcT = stp.tile([128, 32, 32], bf16, tag="rcT")
            stat = cnnp.tile([128, 16], f32, tag="stat")
            stat2 = cnnp.tile([128, 2], f32, tag="stat2")
            stat2r = cnnp.tile([128, 2], f32r, tag="stat2r")
            p32s = cnnp.tile([32, 2], f32r, tag="p32s")
            sdma = cnnp.tile([2, 32], f32, tag="sdma")
            st2 = cnnp.tile([2, 32], f32r, tag="st2")
            agg = cnnp.tile([2, 8, 32], f32, tag="agg")

            A1 = cnnp.tile([128, 4, 2050], bf16, tag="A1")
            A2 = cnnp.tile([128, 4, 2050], bf16, tag="A2")
            SQD = cnnp.tile([128, 2, 512], bf16, tag="SQD")

            nc.sync.dma_start(y_t[:], Y_d[:])
            nc.vector.memset(u_t[:].bitcast(f32), 0.0)
            nc.vector.tensor_copy(zmu_t[:], y_t[:])
            nc.vector.memset(XB[:, 0:1].bitcast(dt.uint16), 0)
            nc.vector.memset(XB[:, 2049:2050].bitcast(dt.uint16), 0)
            for a in (A1, A2):
                nc.vector.memset(a[:, :, 0:1].bitcast(dt.uint16), 0)
                nc.vector.memset(a[:, :, 2049:2050].bitcast(dt.uint16), 0)

            # ---- x0 = A^H y ----
            for ntb in range(4):
                p = psgen.tile([32, 512], f32, tag="mm")
                for k in range(8):
                    nc.tensor.matmul(p[:], ylh[:, k, :],
                                     ab[:, k, 512 * ntb:512 * (ntb + 1)],
                                     start=(k == 0), stop=(k == 7))
                nc.vector.tensor_copy(X[:, 512 * ntb:512 * (ntb + 1)], p[:])

            def combo_pair(dst, slot_top, slot_bot, src_view, idj):
                # plain matmul: out = src^T @ [I|J] (is_transpose ignores the
                # identity's values, so signed J requires the normal datapath)
                pT = pssm.tile([128, 64], f32, tag="sm")
                nc.tensor.matmul(pT[:], src_view, idj, start=True, stop=True)
                nc.vector.tensor_copy(dst[:, slot_top, :], pT[:, 0:32])
                nc.vector.tensor_copy(dst[:, slot_bot, :], pT[:, 32:64])

            def bn_collective(layer, acts):
                with nc.allow_low_precision(reason="bn stat reduce"):
                    nc.vector.tensor_reduce(stat2[:, 0:1], stat[:, 0:8], AX.X, OP.add)
                    nc.vector.tensor_reduce(stat2[:, 1:2], stat[:, 8:16], AX.X, OP.add)
                nc.vector.tensor_copy(stat2r[:], stat2[:])
                p = pssm.tile([32, 2], f32, tag="sm")
                nc.tensor.matmul(p[:], OSEL, stat2r[:], start=True, stop=True)
                with nc.allow_low_precision(reason="pack"):
                    nc.vector.tensor_copy(p32s[:], p[:])
                pt = pssm.tile([2, 32], f32, tag="sm")
                nc.tensor.matmul(pt[:], p32s[:], ID32, start=True, stop=True)
                nc.vector.tensor_copy(sdma[:], pt[:])
                ci_ = ddr.tile([2, 32], f32, tag="cc")
                if USE_AG:
                    co_ = ddr.tile([8, 2, 32], f32, tag="cc2")
                    nc.sync.dma_start(ci_[:], sdma[:])
                    nc.gpsimd.collective_compute(
                        "AllGather", OP.bypass,
                        replica_groups=[list(range(NCORE))],
                        ins=[ci_.opt()], outs=[co_.opt()])
                    nc.sync.dma_start(agg[:], co_[:].rearrange("c s v -> s c v"))
                    with nc.allow_low_precision(reason="bn stat reduce"):
                        nc.vector.tensor_reduce(
                            st2[:], agg[:].rearrange("s c v -> s v c"),
                            AX.X, OP.add)
                else:
                    co1 = ddr.tile([2, 32], f32, tag="cc3")
                    nc.sync.dma_start(ci_[:], sdma[:])
                    nc.gpsimd.collective_compute(
                        "AllReduce", OP.add,
                        replica_groups=[list(range(NCORE))],
                        ins=[ci_.opt()], outs=[co1.opt()])
                    nc.sync.dma_start(st2[:].bitcast(f32), co1[:])
                pb = pssm.tile([32, 32], f32, tag="sm")
                nc.tensor.matmul(pb[:, 0:2], st2[:], ID32[0:2, 0:2],
                                 start=True, stop=True)
                with nc.allow_low_precision(reason="bn scalar math in f32r"):
                    nc.vector.tensor_scalar_mul(smal[:, 0:1], pb[:, 0:1], 1.0 / cnt)
                    nc.vector.tensor_scalar_mul(smal[:, 1:2], pb[:, 1:2], 1.0 / cnt)
                    nc.vector.tensor_mul(smal[:, 2:3], smal[:, 0:1], smal[:, 0:1])
                    nc.vector.tensor_sub(smal[:, 3:4], smal[:, 1:2], smal[:, 2:3])
                    nc.scalar.activation(smal[:, 3:4], smal[:, 3:4], AF.Sqrt,
                                         bias=epsb)
                    nc.vector.reciprocal(smal[:, 3:4], smal[:, 3:4])
                    nc.vector.tensor_mul(gbt[:, 0:1], g32[layer], smal[:, 3:4])
                    nc.vector.tensor_mul(smal[:, 2:3], smal[:, 0:1], gbt[:, 0:1])
                    nc.vector.tensor_sub(gbt[:, 1:2], b32[layer], smal[:, 2:3])
                p2 = pssm.tile([128, 2], f32, tag="sm")
                nc.tensor.matmul(p2[:], SELB, gbt[:], start=True, stop=True)
                nc.vector.tensor_copy(gbb[:], p2[:])
                for bp in range(4):
                    nc.scalar.activation(acts[:, bp, 1:2049], acts[:, bp, 1:2049],
                                         AF.Relu, bias=gbb[:, 1:2],
                                         scale=gbb[:, 0:1])

            # ================= iterations =================
            for it in range(ITERS):
                rho = float(rhos[it])
                eps = float(epss[it])
                c1 = 1.0 / (rho + 1e-8)
                uidx = prep['iter_minv_idx'][it]
                if nu > 1:
                    nc.sync.dma_start(minv[:], MINV_d[uidx])

                # ---- CNN: xb cast + gather ----
                nc.vector.tensor_copy(XB[:, 1:2049], X[:])
                xins = []
                for bp in range(4):
                    xin = xinp.tile([24, 2048], bf16, tag="xin")
                    for dl in range(3):
                        for ci in range(2):
                            eng = nc.sync if (dl % 2 == 0) else nc.scalar
                            eng.dma_start(
                                xin[dl * 8 + ci * 4: dl * 8 + ci * 4 + 4, :],
                                XB[ci * 16 + bp * 4: ci * 16 + bp * 4 + 4,
                                   dl:dl + 2048])
                    xins.append(xin)
                # conv1
                for lt in range(4):
                    for bph in range(2):
                        pb_ = psbig.tile([128, 2, 512], f32, tag="big")
                        for h in range(2):
                            nc.tensor.matmul(
                                pb_[:, h, :], W1,
                                xins[bph * 2 + h][0:24, 512 * lt:512 * (lt + 1)],
                                start=True, stop=True)
                        g = lt * 2 + bph
                        a1v = A1[:, 2 * bph:2 * bph + 2,
                                 1 + 512 * lt:1 + 512 * (lt + 1)]
                        nc.scalar.activation(a1v, pb_[:], AF.Identity,
                                             bias=zb128,
                                             accum_out=stat[:, g:g + 1])
                        nc.vector.scalar_tensor_tensor(
                            SQD[:], a1v, 1.0, a1v, OP.mult, OP.mult,
                            accum_out=stat[:, 8 + g:9 + g])

                # step-0 mz: runs on PE during the bn1 collective stall
                lhs0 = yln if it == 0 else zmuT
                pmz0 = psgen.tile([32, 512], f32, tag="mm")
                for k in range(8):
                    nc.tensor.matmul(pmz0[:], lhs0[:, k, :], minv[:, k, :],
                                     start=(k == 0), stop=(k == 7))
                nc.vector.tensor_copy(mzS[:], pmz0[:])

                bn_collective(0, A1)

                # conv2 (+ early AT stream issue)
                ats = []
                for gidx in range(16):
                    att = atp.tile([128, 2, 512], bf16, tag="at")
                    nc.sync.dma_start(att[:], ATG_d[gidx])
                    ats.append(att)
                for lt in range(4):
                    for bph in range(2):
                        pb_ = psbig.tile([128, 2, 512], f32, tag="big")
                        for h in range(2):
                            for dl in range(3):
                                nc.tensor.matmul(
                                    pb_[:, h, :], W2[dl],
                                    A1[:, 2 * bph + h,
                                       dl + 512 * lt: dl + 512 * lt + 512],
                                    start=(dl == 0), stop=(dl == 2))
                        g = lt * 2 + bph
                        a2v = A2[:, 2 * bph:2 * bph + 2,
                                 1 + 512 * lt:1 + 512 * (lt + 1)]
                        nc.scalar.activation(a2v, pb_[:], AF.Identity,
                                             bias=zb128,
                                             accum_out=stat[:, g:g + 1])
                        nc.vector.scalar_tensor_tensor(
                            SQD[:], a2v, 1.0, a2v, OP.mult, OP.mult,
                            accum_out=stat[:, 8 + g:9 + g])
                bn_collective(1, A2)

                # convf + rc1 + comboN(rc1) + arc matmuls
                parc = psarc.tile([32, 512], f32, tag="arc")
                first_mm = True
                for lt in range(4):
                    pcf = psgen.tile([32, 512], f32, tag="mm")
                    nmm = 0
                    for bp in range(4):
                        for dl in range(3):
                            nc.tensor.matmul(
                                pcf[:], WF[bp][dl],
                                A2[:, bp, dl + 512 * lt: dl + 512 * lt + 512],
                                start=(nmm == 0), stop=(nmm == 11))
                            nmm += 1
                    blk = slice(512 * lt, 512 * (lt + 1))
                    # rc1 = (pcf + x)*c1 + fb*c1
                    nc.vector.scalar_tensor_tensor(
                        sq_t[:], pcf[:], 1.0, X[:, blk], OP.mult, OP.add)
                    nc.scalar.activation(RC[:, blk], sq_t[:], AF.Identity,
                                         bias=fbc[0:32, uidx:uidx + 1], scale=c1)
                    for c in range(4):
                        combo_pair(rcT, lt * 4 + c, 16 + lt * 4 + c,
                                   RC[:, 512 * lt + 128 * c:
                                      512 * lt + 128 * (c + 1)], IDJN)
                    for h in range(2):
                        att = ats[4 * lt + h]
                        for j in range(2):
                            nc.tensor.matmul(parc[:],
                                             rcT[:, lt * 4 + 2 * h + j, :],
                                             att[:, j, :],
                                             start=first_mm, stop=False)
                            first_mm = False
                    for h in range(2):
                        atb = ats[4 * lt + 2 + h]
                        for j in range(2):
                            last = (lt == 3 and h == 1 and j == 1)
                            nc.tensor.matmul(parc[:],
                                             rcT[:, 16 + lt * 4 + 2 * h + j, :],
                                             atb[:, j, :],
                                             start=False, stop=last)
                nc.vector.tensor_copy(arcS[:], parc[:])
                # comboN(arc) -> wT scratch; marc = Minv @ arc
                for c in range(4):
                    combo_pair(wT, c, 4 + c, arcS[:, 128 * c:128 * (c + 1)], IDJN)
                pma = psgen.tile([32, 512], f32, tag="mm")
                for k in range(8):
                    nc.tensor.matmul(pma[:], wT[:, k, :], minv[:, k, :],
                                     start=(k == 0), stop=(k == 7))
                nc.vector.tensor_copy(marc_t[:], pma[:])

                # ---------- ADMM steps ----------
                r2c1 = rho * rho * c1
                r3c1 = rho * rho * rho * c1
                for s in range(ADMM):
                    final = (s == ADMM - 1)
                    nc.vector.scalar_tensor_tensor(
                        t1_t[:], zmu_t[:], r2c1, u_t[:], OP.mult, OP.add)
                    nc.vector.scalar_tensor_tensor(
                        t1_t[:], marc_t[:], rho, t1_t[:], OP.mult, OP.add)
                    nc.vector.tensor_sub(q3_t[:], t1_t[:], y_t[:])
                    if s == 0:
                        mz_ap = mzS[:]
                    else:
                        pmz = psgen.tile([32, 512], f32, tag="mm")
                        for k in range(8):
                            nc.tensor.matmul(pmz[:], zmuT[:, k, :], minv[:, k, :],
                                             start=(k == 0), stop=(k == 7))
                        mz_ap = pmz[:]
                    nc.vector.scalar_tensor_tensor(
                        dif_t[:], mz_ap, -r3c1, q3_t[:], OP.mult, OP.add)
                    if final:
                        nc.vector.scalar_tensor_tensor(
                            t1_t[:], mz_ap, r2c1, marc_t[:], OP.mult, OP.subtract)
                    nc.vector.scalar_tensor_tensor(
                        sq_t[:], dif_t[:], 1.0, dif_t[:], OP.mult, OP.mult,
                        accum_out=s32f[:])
                    pn = pssm.tile([32, 1], f32, tag="sm")
                    nc.tensor.matmul(pn[:], P32.bitcast(f32), s32f[:],
                                     start=True, stop=True)
                    nc.scalar.activation(facs[:, 0:1], pn[:], AF.Sqrt, bias=zb32)
                    nc.vector.reciprocal(facs[:, 0:1], facs[:, 0:1])
                    nc.vector.tensor_scalar(facs[:, 1:2], facs[:, 0:1],
                                            eps, 1.0, OP.mult, OP.min)
                    nc.vector.tensor_scalar(facs[:, 2:3], facs[:, 1:2],
                                            -1.0, 1.0, OP.mult, OP.add)
                    if final:
                        nc.vector.tensor_scalar(facs[:, 3:4], facs[:, 1:2],
                                                1.0, -1.0, OP.mult, OP.add)
                    else:
                        nc.vector.tensor_scalar(facs[:, 3:4], facs[:, 1:2],
                                                2.0, -1.0, OP.mult, OP.add)
                    last_all = (it == ITERS - 1 and final)
                    if not last_all:
                        nc.vector.scalar_tensor_tensor(
                            zmu_t[:], dif_t[:], facs[:, 3:4], y_t[:],
                            OP.mult, OP.add)
                        nc.vector.tensor_scalar(u_t[:], dif_t[:], facs[:, 2:3],
                                                None, OP.mult)
                        for c in range(4):
                            combo_pair(zmuT, c, 4 + c,
                                       zmu_t[:, 128 * c:128 * (c + 1)], IDJN)
                    if final:
                        for c in range(4):
                            combo_pair(wT, c, 4 + c,
                                       t1_t[:, 128 * c:128 * (c + 1)], IDJH)
                        for ntb in range(4):
                            px = psgen.tile([32, 512], f32, tag="mm")
                            for k in range(8):
                                nc.tensor.matmul(
                                    px[:], wT[:, k, :],
                                    ab[:, k, 512 * ntb:512 * (ntb + 1)],
                                    start=(k == 0), stop=(k == 7))
                            blk = slice(512 * ntb, 512 * (ntb + 1))
                            nc.vector.tensor_add(X[:, blk], px[:], RC[:, blk])

            nc.sync.dma_start(XO_d[:], X[:])

    nc.compile()
    return nc


def _enable_trace_shim():
    import sys, types
    try:
        import trn_agent_boot.trn_boot as _tb
        import concourse.bass_utils as _bu
        _bu.upload_artifacts = lambda tmpdir: "local://" + str(tmpdir)
        hookmod = types.ModuleType('antenv.axon_hooks')
        hook = _tb._ntff_profile_via_ctypes('/opt/axon/libaxon_pjrt.so')
        hookmod.get_axon_ntff_profile_hook = lambda: hook
        import antenv as _antenv
        sys.modules['antenv.axon_hooks'] = hookmod
        _antenv.axon_hooks = hookmod
        return True
    except Exception:
        return False


def kernel(**inputs) -> np.ndarray:
    import os
    from concourse.bass_utils import run_bass_kernel_spmd
    trace = bool(os.environ.get("KERNEL_TRACE"))
    if trace:
        trace = _enable_trace_shim()

    prep = _host_prep(inputs)
    nc = _build_program(prep)

    minvs = np.stack(prep['minv_stacks'], 0)
    in_maps = []
    for c in range(NCORE):
        in_maps.append({
            "AB": prep['AB'], "ATG": prep['ATG'], "MINVS": minvs,
            "WTS": prep['WTS'], "WTB": prep['WTB'], "CF": prep['CF'],
            "FBC": prep['FBC'],
            "YBM": np.ascontiguousarray(prep['ybm_cores'][c][:, :512]),
            "YLH": prep['ylh_cores'][c], "YLN": prep['yln_cores'][c],
        })
    res = run_bass_kernel_spmd(nc, in_maps, list(range(NCORE)), trace=trace)
    out = np.zeros((B, 2, Nt), np.float32)
    for c in range(NCORE):
        xc = res.results[c]["XOUT"]
        out[c * BS:(c + 1) * BS, 0] = xc[:16]
        out[c * BS:(c + 1) * BS, 1] = xc[16:]
    kernel._last_results = res
    return out


# revision 11
# speedup vs baseline: 1.8634x; 1.0538x over previous
"""DBPNet Trainium2 kernel: 8-core data-parallel Bass/Tile implementation.

v2 scheme:
  - ADMM algebra: with M = (AAH + rho I)^-1,  AAH@M = I - rho*M, so
      tmv = marc + rho*c1*zmu - rho^2*c1*mz      (mz = M@zmu, marc = M@arc)
      Ax  = rho*tmv   (no AAH matmuls on device at all)
      x   = rc1 + A^H(rho^2*c1*mz - marc)
    -> one 8-matmul group (mz) per ADMM step.
  - proj state: u' = (1-fac)*diff; zmu'(within iter) = y + (2fac-1)*diff;
    zmu0(next iter) = y + (fac-1)*diff.
  - combo transposes via wide signed identities [I|J]: one PE op -> top+bottom.
  - CNN in bf16 single-tile activations [128, 4bp, 2050], block-diag weights.
  - arc path (AT stream + rcT) in bf16; everything else f32r.
  - BN stats pre-reduced to [32,2] and packed [2,32] before the collective
    (2/16-descriptor DMAs); AllGather + local reduce.
"""
import numpy as np

B, Nv, Nt, F = 128, 512, 2048, 32
NCORE, BS = 8, 16
ITERS, ADMM = 5, 3
BN_EPS = 1e-5
USE_AG = True


# ---------------------------------------------------------------- host prep
def _host_prep(inputs):
    import ml_dtypes
    A = np.ascontiguousarray(np.asarray(inputs['A'], np.float32))
    Ar, Ai = A[0], A[1]
    Ac = Ar.astype(np.float64) + 1j * Ai.astype(np.float64)
    AAH = Ac @ Ac.conj().T

    rhos = np.exp(np.asarray(inputs['log_rho'], np.float32)).astype(np.float32)
    epss = np.exp(np.asarray(inputs['log_eps'], np.float32)).astype(np.float32)

    minv_stacks, rho_to_idx, iter_minv_idx = [], {}, []
    for r in rhos:
        key = float(r)
        if key not in rho_to_idx:
            M = np.linalg.inv(AAH + key * np.eye(Nv))
            Mr = M.real.astype(np.float32)
            Mi = M.imag.astype(np.float32)
            minv_stacks.append(
                np.concatenate([Mr.T, Mi.T], 0).reshape(8, 128, 512)
                .transpose(1, 0, 2).copy().astype(ml_dtypes.bfloat16))
            rho_to_idx[key] = len(minv_stacks) - 1
        iter_minv_idx.append(rho_to_idx[float(r)])
    nu = len(minv_stacks)

    A1 = np.concatenate([Ar, Ai], 0)                    # [1024, 2048]
    AB = A1.reshape(8, 128, 2048).transpose(1, 0, 2).copy().astype(
        ml_dtypes.bfloat16)                             # [128, 8, 2048] bf16
    AT1 = np.concatenate([Ar.T, Ai.T], 0)               # [4096, 512]
    ATR = AT1.reshape(32, 128, 512).transpose(1, 0, 2).copy().astype(
        ml_dtypes.bfloat16)                             # [128, 32, 512] bf16

    # bf16 CNN weights pack [128, 896]
    w1 = np.asarray(inputs['conv1_w'], np.float32)
    w2 = np.asarray(inputs['conv2_w'], np.float32)
    wf = np.asarray(inputs['convf_w'], np.float32)
    W1 = np.zeros((128, 128), np.float32)
    for dl in range(3):
        for ci in range(2):
            for q in range(4):
                W1[dl * 8 + ci * 4 + q, np.arange(F) * 4 + q] = w1[:, ci, dl]
    W2 = np.zeros((3, 128, 128), np.float32)
    for dl in range(3):
        for ci in range(F):
            for q in range(4):
                W2[dl, ci * 4 + q, np.arange(F) * 4 + q] = w2[:, ci, dl]
    WFb = np.zeros((4, 3, 128, 32), np.float32)
    for bp in range(4):
        for dl in range(3):
            for ci in range(F):
                for q in range(4):
                    for cf in range(2):
                        WFb[bp, dl, ci * 4 + q, cf * 16 + bp * 4 + q] = wf[cf, ci, dl]
    WTB = np.concatenate(
        [W1] + [W2[d] for d in range(3)]
        + [WFb[bp, dl] for bp in range(4) for dl in range(3)],
        axis=1).astype(ml_dtypes.bfloat16)              # [128, 896]

    # f32 selector/identity pack
    OSEL = np.zeros((128, 32), np.float32)
    SELB = np.zeros((128, 128), np.float32)   # rows 0-31 used
    for co in range(32):
        for q in range(4):
            OSEL[co * 4 + q, co] = 1.0
            SELB[co, co * 4 + q] = 1.0
    I16 = np.eye(16, dtype=np.float32)
    JN = np.zeros((32, 32), np.float32)
    JN[16:, :16] = -I16
    JN[:16, 16:] = I16
    JH = -JN
    ID32 = np.eye(32, dtype=np.float32)
    IDJN = np.zeros((128, 64), np.float32)
    IDJN[:32, :32] = ID32
    IDJN[:32, 32:] = JN
    IDJH = np.zeros((128, 64), np.float32)
    IDJH[:32, :32] = ID32
    IDJH[:32, 32:] = JH
    P32 = np.zeros((128, 32), np.float32)
    for m in range(32):
        P32[m, m] = 1.0
        P32[(m + 16) % 32, m] = 1.0
    ID32p = np.zeros((128, 32), np.float32)
    ID32p[:32, :32] = ID32
    WTS = np.concatenate([OSEL, SELB, IDJN, IDJH, P32, ID32p], axis=1)

    g1 = np.asarray(inputs['bn1_g'], np.float32)
    b1 = np.asarray(inputs['bn1_b'], np.float32)
    g2 = np.asarray(inputs['bn2_g'], np.float32)
    b2 = np.asarray(inputs['bn2_b'], np.float32)
    fb = np.asarray(inputs['convf_b'], np.float32)
    CF = np.zeros((128, 8), np.float32)
    CF[:32, 0] = g1
    CF[:32, 1] = b1
    CF[:32, 2] = g2
    CF[:32, 3] = b2
    CF[:, 6] = BN_EPS
    FBC = np.zeros((128, nu), np.float32)
    for key, idx in rho_to_idx.items():
        c1 = 1.0 / (key + 1e-8)
        FBC[:16, idx] = fb[0] * c1
        FBC[16:32, idx] = fb[1] * c1

    y = np.asarray(inputs['y'], np.float32)
    ybm_cores, ylh_cores, yln_cores = [], [], []
    for c in range(NCORE):
        ys = y[c * BS:(c + 1) * BS]
        ybm = np.concatenate([ys[:, 0], ys[:, 1]], 0)    # [32, Nv]
        ybm_cores.append(np.ascontiguousarray(ybm))
        sT = ybm.T                                       # [Nv, 32]
        botH = np.concatenate([sT[:, 16:], -sT[:, :16]], 1)
        botN = np.concatenate([-sT[:, 16:], sT[:, :16]], 1)
        comboH = np.concatenate([sT, botH], 0)           # [2Nv, 32]
        comboN = np.concatenate([sT, botN], 0)
        ylh_cores.append(comboH.reshape(8, 128, 32).transpose(1, 0, 2)
                         .copy().astype(ml_dtypes.bfloat16))
        yln_cores.append(comboN.reshape(8, 128, 32).transpose(1, 0, 2)
                         .copy().astype(ml_dtypes.bfloat16))

    return dict(AB=AB, ATR=ATR, minv_stacks=minv_stacks,
                iter_minv_idx=iter_minv_idx, rhos=rhos, epss=epss,
                WTS=WTS, WTB=WTB, CF=CF, FBC=FBC, ybm_cores=ybm_cores,
                ylh_cores=ylh_cores, yln_cores=yln_cores)


# WTS column offsets
OSEL_C = 0
SELB_C = 32
IDJN_C = SELB_C + 128
IDJH_C = IDJN_C + 64
P32_C = IDJH_C + 64
ID32_C = P32_C + 32
WTS_W = ID32_C + 32
# WTB column offsets
W1_C = 0
W2_C = 128
WF_C = W2_C + 384
WTB_W = WF_C + 384


# ---------------------------------------------------------------- program
def _build_program(prep):
    import concourse.bacc as bacc
    import concourse.tile as tile
    import concourse.mybir as mybir

    dt = mybir.dt
    f32, f32r, bf16 = dt.float32, dt.float32r, dt.bfloat16
    AX = mybir.AxisListType
    OP = mybir.AluOpType
    AF = mybir.ActivationFunctionType

    nu = len(prep['minv_stacks'])
    rhos, epss = prep['rhos'], prep['epss']
    cnt = float(B * Nt)

    nc = bacc.Bacc("TRN2", target_bir_lowering=False, debug=False,
                   num_devices=NCORE)

    AB_d = nc.dram_tensor("AB", [128, 8, 2048], bf16, kind="ExternalInput")
    ATR_d = nc.dram_tensor("ATR", [128, 32, 512], bf16, kind="ExternalInput")
    MINV_d = nc.dram_tensor("MINVS", [nu, 128, 8, 512], bf16, kind="ExternalInput")
    WTS_d = nc.dram_tensor("WTS", [128, WTS_W], f32r, kind="ExternalInput")
    WTB_d = nc.dram_tensor("WTB", [128, WTB_W], bf16, kind="ExternalInput")
    CF_d = nc.dram_tensor("CF", [128, 8], f32, kind="ExternalInput")
    FBC_d = nc.dram_tensor("FBC", [128, nu], f32, kind="ExternalInput")
    Y_d = nc.dram_tensor("YBM", [32, 512], f32r, kind="ExternalInput")
    YLH_d = nc.dram_tensor("YLH", [128, 8, 32], bf16, kind="ExternalInput")
    YLN_d = nc.dram_tensor("YLN", [128, 8, 32], bf16, kind="ExternalInput")
    XO_d = nc.dram_tensor("XOUT", [32, 2048], f32r, kind="ExternalOutput")

    with tile.TileContext(nc) as tc:
        with (
            tc.tile_pool(name="cst", bufs=1) as cst,
            tc.tile_pool(name="st", bufs=1) as stp,
            tc.tile_pool(name="cnn", bufs=1) as cnnp,
            tc.tile_pool(name="xin", bufs=4) as xinp,
            tc.tile_pool(name="psbig", bufs=2, space="PSUM") as psbig,
            tc.tile_pool(name="psgen", bufs=2, space="PSUM") as psgen,
            tc.tile_pool(name="psarc", bufs=1, space="PSUM") as psarc,
            tc.tile_pool(name="pssm", bufs=1, space="PSUM") as pssm,
            tc.tile_pool(name="ddr", bufs=2, space="DRAM") as ddr,
        ):
            # ---- constants ----
            ab = cst.tile([128, 8, 2048], bf16, tag="ab")
            atr = cst.tile([128, 32, 512], bf16, tag="atr")
            minv = cst.tile([128, 8, 512], bf16, tag="minv")
            wts = cst.tile([128, WTS_W], f32r, tag="wts")
            wtb = cst.tile([128, WTB_W], bf16, tag="wtb")
            cf = cst.tile([128, 8], f32, tag="cf")
            fbc = cst.tile([128, nu], f32, tag="fbc")
            ylh = cst.tile([128, 8, 32], bf16, tag="ylh")
            yln = cst.tile([128, 8, 32], bf16, tag="yln")
            nc.sync.dma_start(ab[:], AB_d[:])
            nc.sync.dma_start(atr[:], ATR_d[:])
            nc.sync.dma_start(wts[:], WTS_d[:])
            nc.sync.dma_start(wtb[:], WTB_d[:])
            nc.sync.dma_start(cf[:], CF_d[:])
            nc.sync.dma_start(fbc[:], FBC_d[:])
            nc.sync.dma_start(ylh[:], YLH_d[:])
            nc.sync.dma_start(yln[:], YLN_d[:])
            if nu == 1:
                nc.sync.dma_start(minv[:], MINV_d[0])

            OSEL = wts[:, OSEL_C:OSEL_C + 32]
            SELB = wts[0:32, SELB_C:SELB_C + 128]
            IDJN = wts[0:32, IDJN_C:IDJN_C + 64]
            IDJH = wts[0:32, IDJH_C:IDJH_C + 64]
            P32 = wts[0:32, P32_C:P32_C + 32]
            ID32 = wts[0:32, ID32_C:ID32_C + 32]
            W1 = wtb[0:24, W1_C:W1_C + 128]
            W2 = [wtb[:, W2_C + 128 * d: W2_C + 128 * (d + 1)] for d in range(3)]
            WF = [[wtb[:, WF_C + (bp * 3 + d) * 32: WF_C + (bp * 3 + d) * 32 + 32]
                   for d in range(3)] for bp in range(4)]
            g32 = [cf[0:32, 0:1], cf[0:32, 2:3]]
            b32 = [cf[0:32, 1:2], cf[0:32, 3:4]]
            zb128 = cf[:, 5:6]
            zb32 = cf[0:32, 5:6]
            epsb = cf[0:32, 6:7]

            # ---- state ----
            X = stp.tile([32, 2048], f32r, tag="X")
            RC = stp.tile([32, 2048], f32r, tag="RC")
            XB = stp.tile([32, 2050], bf16, tag="XB")
            S = stp.tile([32, 8, 512], f32r, tag="S")
            y_t, u_t, zmu_t, marc_t = S[:, 0, :], S[:, 1, :], S[:, 2, :], S[:, 3, :]
            q3_t, t1_t, dif_t, sq_t = S[:, 4, :], S[:, 5, :], S[:, 6, :], S[:, 7, :]
            arcS = stp.tile([32, 512], f32r, tag="arcS")
            mzS = stp.tile([32, 512], f32r, tag="mzS")
            s32f = stp.tile([32, 1], f32, tag="s32f")
            facs = stp.tile([32, 4], f32, tag="facs")
            smal = stp.tile([32, 8], f32r, tag="smal")
            gbt = stp.tile([32, 2], f32r, tag="gbt")
            gbb = stp.tile([128, 2], f32, tag="gbb")
            zmuT = stp.tile([128, 8, 32], bf16, tag="zmuT")
            wT = stp.tile([128, 8, 32], bf16, tag="wT")
            rcT = stp.tile([128, 32, 32], bf16, tag="rcT")
            stat = cnnp.tile([128, 24], f32, tag="stat")
            stat2 = cnnp.tile([128, 2], f32, tag="stat2")
            stat2r = cnnp.tile([128, 2], f32r, tag="stat2r")
            p32s = cnnp.tile([32, 2], f32r, tag="p32s")
            sdma = cnnp.tile([2, 32], f32, tag="sdma")
            st2 = cnnp.tile([2, 32], f32r, tag="st2")
            agg = cnnp.tile([2, 8, 32], f32, tag="agg")

            A1 = cnnp.tile([128, 4, 2050], bf16, tag="A1")
            A2 = cnnp.tile([128, 4, 2050], bf16, tag="A2")
            SQD = cnnp.tile([128, 2, 512], bf16, tag="SQD")

            nc.sync.dma_start(y_t[:], Y_d[:])
            nc.vector.memset(u_t[:].bitcast(f32), 0.0)
            nc.vector.tensor_copy(zmu_t[:], y_t[:])
            nc.vector.memset(XB[:, 0:1].bitcast(dt.uint16), 0)
            nc.vector.memset(XB[:, 2049:2050].bitcast(dt.uint16), 0)
            for a in (A1, A2):
                nc.vector.memset(a[:, :, 0:1].bitcast(dt.uint16), 0)
                nc.vector.memset(a[:, :, 2049:2050].bitcast(dt.uint16), 0)

            # ---- x0 = A^H y ----
            for ntb in range(4):
                p = psgen.tile([32, 512], f32, tag="mm")
                for k in range(8):
                    nc.tensor.matmul(p[:], ylh[:, k, :],
                                     ab[:, k, 512 * ntb:512 * (ntb + 1)],
                                     start=(k == 0), stop=(k == 7))
                nc.vector.tensor_copy(X[:, 512 * ntb:512 * (ntb + 1)], p[:])

            def combo_pair(dst, slot_top, slot_bot, src_view, idj):
                # plain matmul: out = src^T @ [I|J] (is_transpose ignores the
                # identity's values, so signed J requires the normal datapath)
                pT = pssm.tile([128, 64], f32, tag="sm")
                nc.tensor.matmul(pT[:], src_view, idj, start=True, stop=True)
                nc.vector.tensor_copy(dst[:, slot_top, :], pT[:, 0:32])
                nc.vector.tensor_copy(dst[:, slot_bot, :], pT[:, 32:64])

            def bn_collective(layer, acts):
                with nc.allow_low_precision(reason="bn stat reduce"):
                    nc.vector.tensor_reduce(stat2[:, 0:1], stat[:, 0:8], AX.X, OP.add)
                    nc.vector.tensor_reduce(stat2[:, 1:2], stat[:, 8:24], AX.X, OP.add)
                nc.vector.tensor_copy(stat2r[:], stat2[:])
                p = pssm.tile([32, 2], f32, tag="sm")
                nc.tensor.matmul(p[:], OSEL, stat2r[:], start=True, stop=True)
                with nc.allow_low_precision(reason="pack"):
                    nc.vector.tensor_copy(p32s[:], p[:])
                pt = pssm.tile([2, 32], f32, tag="sm")
                nc.tensor.matmul(pt[:], p32s[:], ID32, start=True, stop=True)
                nc.vector.tensor_copy(sdma[:], pt[:])
                ci_ = ddr.tile([2, 32], f32, tag="cc")
                if USE_AG:
                    co_ = ddr.tile([8, 2, 32], f32, tag="cc2")
                    nc.sync.dma_start(ci_[:], sdma[:])
                    nc.gpsimd.collective_compute(
                        "AllGather", OP.bypass,
                        replica_groups=[list(range(NCORE))],
                        ins=[ci_.opt()], outs=[co_.opt()])
                    nc.sync.dma_start(agg[:], co_[:].rearrange("c s v -> s c v"))
                    with nc.allow_low_precision(reason="bn stat reduce"):
                        nc.vector.tensor_reduce(
                            st2[:], agg[:].rearrange("s c v -> s v c"),
                            AX.X, OP.add)
                else:
                    co1 = ddr.tile([2, 32], f32, tag="cc3")
                    nc.sync.dma_start(ci_[:], sdma[:])
                    nc.gpsimd.collective_compute(
                        "AllReduce", OP.add,
                        replica_groups=[list(range(NCORE))],
                        ins=[ci_.opt()], outs=[co1.opt()])
                    nc.sync.dma_start(st2[:].bitcast(f32), co1[:])
                pb = pssm.tile([32, 32], f32, tag="sm")
                nc.tensor.matmul(pb[:, 0:2], st2[:], ID32[0:2, 0:2],
                                 start=True, stop=True)
                with nc.allow_low_precision(reason="bn scalar math in f32r"):
                    nc.vector.tensor_scalar_mul(smal[:, 0:1], pb[:, 0:1], 1.0 / cnt)
                    nc.vector.tensor_scalar_mul(smal[:, 1:2], pb[:, 1:2], 1.0 / cnt)
                    nc.vector.tensor_mul(smal[:, 2:3], smal[:, 0:1], smal[:, 0:1])
                    nc.vector.tensor_sub(smal[:, 3:4], smal[:, 1:2], smal[:, 2:3])
                    nc.scalar.activation(smal[:, 3:4], smal[:, 3:4], AF.Sqrt,
                                         bias=epsb)
                    nc.vector.reciprocal(smal[:, 3:4], smal[:, 3:4])
                    nc.vector.tensor_mul(gbt[:, 0:1], g32[layer], smal[:, 3:4])
                    nc.vector.tensor_mul(smal[:, 2:3], smal[:, 0:1], gbt[:, 0:1])
                    nc.vector.tensor_sub(gbt[:, 1:2], b32[layer], smal[:, 2:3])
                p2 = pssm.tile([128, 2], f32, tag="sm")
                nc.tensor.matmul(p2[:], SELB, gbt[:], start=True, stop=True)
                nc.vector.tensor_copy(gbb[:], p2[:])
                for bp in range(2):
                    nc.scalar.activation(acts[:, bp, 1:2049], acts[:, bp, 1:2049],
                                         AF.Relu, bias=gbb[:, 1:2],
                                         scale=gbb[:, 0:1])
                for bp in range(2, 4):
                    av = acts[:, bp, 1:2049]
                    nc.vector.tensor_scalar(av, av, gbb[:, 0:1], gbb[:, 1:2],
                                            OP.mult, OP.add)
                    nc.vector.tensor_scalar(av, av, 0.0, None, OP.max)

            # ================= iterations =================
            for it in range(ITERS):
                rho = float(rhos[it])
                eps = float(epss[it])
                c1 = 1.0 / (rho + 1e-8)
                uidx = prep['iter_minv_idx'][it]
                if nu > 1:
                    nc.sync.dma_start(minv[:], MINV_d[uidx])

                # ---- CNN: xb cast + gather ----
                nc.vector.tensor_copy(XB[:, 1:2049], X[:])
                xins = []
                for bp in range(4):
                    xin = xinp.tile([24, 2048], bf16, tag="xin")
                    for dl in range(3):
                        for ci in range(2):
                            eng = nc.sync if (dl % 2 == 0) else nc.scalar
                            eng.dma_start(
                                xin[dl * 8 + ci * 4: dl * 8 + ci * 4 + 4, :],
                                XB[ci * 16 + bp * 4: ci * 16 + bp * 4 + 4,
                                   dl:dl + 2048])
                    xins.append(xin)
                # conv1
                for lt in range(4):
                    for bph in range(2):
                        pb_ = psbig.tile([128, 2, 512], f32, tag="big")
                        for h in range(2):
                            nc.tensor.matmul(
                                pb_[:, h, :], W1,
                                xins[bph * 2 + h][0:24, 512 * lt:512 * (lt + 1)],
                                start=True, stop=True)
                        g = lt * 2 + bph
                        a1v = A1[:, 2 * bph:2 * bph + 2,
                                 1 + 512 * lt:1 + 512 * (lt + 1)]
                        nc.scalar.activation(a1v, pb_[:], AF.Identity,
                                             bias=zb128,
                                             accum_out=stat[:, g:g + 1])
                        for h in range(2):
                            a1v2 = A1[:, 2 * bph + h,
                                      1 + 512 * lt:1 + 512 * (lt + 1)]
                            nc.vector.scalar_tensor_tensor(
                                SQD[:, h, :], a1v2, 1.0, a1v2, OP.mult, OP.mult,
                                accum_out=stat[:, 8 + g:9 + g] if h else
                                stat[:, 16 + g:17 + g])

                # step-0 mz: runs on PE during the bn1 collective stall
                lhs0 = yln if it == 0 else zmuT
                pmz0 = psgen.tile([32, 512], f32, tag="mm")
                for k in range(8):
                    nc.tensor.matmul(pmz0[:], lhs0[:, k, :], minv[:, k, :],
                                     start=(k == 0), stop=(k == 7))
                nc.vector.tensor_copy(mzS[:], pmz0[:])

                bn_collective(0, A1)

                # conv2
                for lt in range(4):
                    for bph in range(2):
                        pb_ = psbig.tile([128, 2, 512], f32, tag="big")
                        for h in range(2):
                            for dl in range(3):
                                nc.tensor.matmul(
                                    pb_[:, h, :], W2[dl],
                                    A1[:, 2 * bph + h,
                                       dl + 512 * lt: dl + 512 * lt + 512],
                                    start=(dl == 0), stop=(dl == 2))
                        g = lt * 2 + bph
                        a2v = A2[:, 2 * bph:2 * bph + 2,
                                 1 + 512 * lt:1 + 512 * (lt + 1)]
                        nc.scalar.activation(a2v, pb_[:], AF.Identity,
                                             bias=zb128,
                                             accum_out=stat[:, g:g + 1])
                        for h in range(2):
                            a2v2 = A2[:, 2 * bph + h,
                                      1 + 512 * lt:1 + 512 * (lt + 1)]
                            nc.vector.scalar_tensor_tensor(
                                SQD[:, h, :], a2v2, 1.0, a2v2, OP.mult, OP.mult,
                                accum_out=stat[:, 8 + g:9 + g] if h else
                                stat[:, 16 + g:17 + g])
                bn_collective(1, A2)

                # convf + rc1 + comboN(rc1) + arc matmuls
                parc = psarc.tile([32, 512], f32, tag="arc")
                first_mm = True
                for lt in range(4):
                    pcf = psgen.tile([32, 512], f32, tag="mm")
                    nmm = 0
                    for bp in range(4):
                        for dl in range(3):
                            nc.tensor.matmul(
                                pcf[:], WF[bp][dl],
                                A2[:, bp, dl + 512 * lt: dl + 512 * lt + 512],
                                start=(nmm == 0), stop=(nmm == 11))
                            nmm += 1
                    blk = slice(512 * lt, 512 * (lt + 1))
                    # rc1 = (pcf + x)*c1 + fb*c1
                    nc.vector.scalar_tensor_tensor(
                        sq_t[:], pcf[:], 1.0, X[:, blk], OP.mult, OP.add)
                    nc.scalar.activation(RC[:, blk], sq_t[:], AF.Identity,
                                         bias=fbc[0:32, uidx:uidx + 1], scale=c1)
                    for c in range(4):
                        combo_pair(rcT, lt * 4 + c, 16 + lt * 4 + c,
                                   RC[:, 512 * lt + 128 * c:
                                      512 * lt + 128 * (c + 1)], IDJN)
                    for c in range(4):
                        nc.tensor.matmul(parc[:], rcT[:, lt * 4 + c, :],
                                         atr[:, lt * 4 + c, :],
                                         start=first_mm, stop=False)
                        first_mm = False
                        last = (lt == 3 and c == 3)
                        nc.tensor.matmul(parc[:], rcT[:, 16 + lt * 4 + c, :],
                                         atr[:, 16 + lt * 4 + c, :],
                                         start=False, stop=last)
                nc.vector.tensor_copy(arcS[:], parc[:])
                # comboN(arc) -> wT scratch; marc = Minv @ arc
                for c in range(4):
                    combo_pair(wT, c, 4 + c, arcS[:, 128 * c:128 * (c + 1)], IDJN)
                pma = psgen.tile([32, 512], f32, tag="mm")
                for k in range(8):
                    nc.tensor.matmul(pma[:], wT[:, k, :], minv[:, k, :],
                                     start=(k == 0), stop=(k == 7))
                nc.vector.tensor_copy(marc_t[:], pma[:])

                # ---------- ADMM steps ----------
                r2c1 = rho * rho * c1
                r3c1 = rho * rho * rho * c1
                for s in range(ADMM):
                    final = (s == ADMM - 1)
                    nc.vector.scalar_tensor_tensor(
                        t1_t[:], zmu_t[:], r2c1, u_t[:], OP.mult, OP.add)
                    nc.vector.scalar_tensor_tensor(
                        t1_t[:], marc_t[:], rho, t1_t[:], OP.mult, OP.add)
                    nc.vector.tensor_sub(q3_t[:], t1_t[:], y_t[:])
                    if s == 0:
                        mz_ap = mzS[:]
                    else:
                        pmz = psgen.tile([32, 512], f32, tag="mm")
                        for k in range(8):
                            nc.tensor.matmul(pmz[:], zmuT[:, k, :], minv[:, k, :],
                                             start=(k == 0), stop=(k == 7))
                        mz_ap = pmz[:]
                    nc.vector.scalar_tensor_tensor(
                        dif_t[:], mz_ap, -r3c1, q3_t[:], OP.mult, OP.add)
                    if final:
                        nc.vector.scalar_tensor_tensor(
                            t1_t[:], mz_ap, r2c1, marc_t[:], OP.mult, OP.subtract)
                    nc.vector.scalar_tensor_tensor(
                        sq_t[:], dif_t[:], 1.0, dif_t[:], OP.mult, OP.mult,
                        accum_out=s32f[:])
                    pn = pssm.tile([32, 1], f32, tag="sm")
                    nc.tensor.matmul(pn[:], P32.bitcast(f32), s32f[:],
                                     start=True, stop=True)
                    nc.scalar.activation(facs[:, 0:1], pn[:], AF.Sqrt, bias=zb32)
                    nc.vector.reciprocal(facs[:, 0:1], facs[:, 0:1])
                    nc.vector.tensor_scalar(facs[:, 1:2], facs[:, 0:1],
                                            eps, 1.0, OP.mult, OP.min)
                    nc.vector.tensor_scalar(facs[:, 2:3], facs[:, 1:2],
                                            -1.0, 1.0, OP.mult, OP.add)
                    if final:
                        nc.vector.tensor_scalar(facs[:, 3:4], facs[:, 1:2],
                                                1.0, -1.0, OP.mult, OP.add)
                    else:
                        nc.vector.tensor_scalar(facs[:, 3:4], facs[:, 1:2],
                                                2.0, -1.0, OP.mult, OP.add)
                    last_all = (it == ITERS - 1 and final)
                    if not last_all:
                        nc.vector.scalar_tensor_tensor(
                            zmu_t[:], dif_t[:], facs[:, 3:4], y_t[:],
                            OP.mult, OP.add)
                        nc.vector.tensor_scalar(u_t[:], dif_t[:], facs[:, 2:3],
                                                None, OP.mult)
                        for c in range(4):
                            combo_pair(zmuT, c, 4 + c,
                                       zmu_t[:, 128 * c:128 * (c + 1)], IDJN)
                    if final:
                        for c in range(4):
                            combo_pair(wT, c, 4 + c,
                                       t1_t[:, 128 * c:128 * (c + 1)], IDJH)
                        for ntb in range(4):
                            px = psgen.tile([32, 512], f32, tag="mm")
                            for k in range(8):
                                nc.tensor.matmul(
                                    px[:], wT[:, k, :],
                                    ab[:, k, 512 * ntb:512 * (ntb + 1)],
                                    start=(k == 0), stop=(k == 7))
                            blk = slice(512 * ntb, 512 * (ntb + 1))
                            nc.vector.tensor_add(X[:, blk], px[:], RC[:, blk])

            nc.sync.dma_start(XO_d[:], X[:])

    nc.compile()
    return nc


def _enable_trace_shim():
    import sys, types
    try:
        import trn_agent_boot.trn_boot as _tb
        import concourse.bass_utils as _bu
        _bu.upload_artifacts = lambda tmpdir: "local://" + str(tmpdir)
        hookmod = types.ModuleType('antenv.axon_hooks')
        hook = _tb._ntff_profile_via_ctypes('/opt/axon/libaxon_pjrt.so')
        hookmod.get_axon_ntff_profile_hook = lambda: hook
        import antenv as _antenv
        sys.modules['antenv.axon_hooks'] = hookmod
        _antenv.axon_hooks = hookmod
        return True
    except Exception:
        return False


def kernel(**inputs) -> np.ndarray:
    import os
    from concourse.bass_utils import run_bass_kernel_spmd
    trace = bool(os.environ.get("KERNEL_TRACE"))
    if trace:
        trace = _enable_trace_shim()

    prep = _host_prep(inputs)
    nc = _build_program(prep)

    minvs = np.stack(prep['minv_stacks'], 0)
    in_maps = []
    for c in range(NCORE):
        in_maps.append({
            "AB": prep['AB'], "ATR": prep['ATR'], "MINVS": minvs,
            "WTS": prep['WTS'], "WTB": prep['WTB'], "CF": prep['CF'],
            "FBC": prep['FBC'],
            "YBM": np.ascontiguousarray(prep['ybm_cores'][c][:, :512]),
            "YLH": prep['ylh_cores'][c], "YLN": prep['yln_cores'][c],
        })
    res = run_bass_kernel_spmd(nc, in_maps, list(range(NCORE)), trace=trace)
    out = np.zeros((B, 2, Nt), np.float32)
    for c in range(NCORE):
        xc = res.results[c]["XOUT"]
        out[c * BS:(c + 1) * BS, 0] = xc[:16]
        out[c * BS:(c + 1) * BS, 1] = xc[16:]
    kernel._last_results = res
    return out


# revision 13
# speedup vs baseline: 2.0440x; 1.0970x over previous
"""DBPNet Trainium2 kernel: 8-core data-parallel Bass/Tile implementation.

v2 scheme:
  - ADMM algebra: with M = (AAH + rho I)^-1,  AAH@M = I - rho*M, so
      tmv = marc + rho*c1*zmu - rho^2*c1*mz      (mz = M@zmu, marc = M@arc)
      Ax  = rho*tmv   (no AAH matmuls on device at all)
      x   = rc1 + A^H(rho^2*c1*mz - marc)
    -> one 8-matmul group (mz) per ADMM step.
  - proj state: u' = (1-fac)*diff; zmu'(within iter) = y + (2fac-1)*diff;
    zmu0(next iter) = y + (fac-1)*diff.
  - combo transposes via wide signed identities [I|J]: one PE op -> top+bottom.
  - CNN in bf16 single-tile activations [128, 4bp, 2050], block-diag weights.
  - arc path (AT stream + rcT) in bf16; everything else f32r.
  - BN stats pre-reduced to [32,2] and packed [2,32] before the collective
    (2/16-descriptor DMAs); AllGather + local reduce.
"""
import numpy as np

B, Nv, Nt, F = 128, 512, 2048, 32
NCORE, BS = 8, 16
ITERS, ADMM = 5, 3
BN_EPS = 1e-5
USE_AG = True


# ---------------------------------------------------------------- host prep
def _host_prep(inputs):
    import ml_dtypes
    A = np.ascontiguousarray(np.asarray(inputs['A'], np.float32))
    Ar, Ai = A[0], A[1]
    Ac = Ar.astype(np.float64) + 1j * Ai.astype(np.float64)
    AAH = Ac @ Ac.conj().T

    rhos = np.exp(np.asarray(inputs['log_rho'], np.float32)).astype(np.float32)
    epss = np.exp(np.asarray(inputs['log_eps'], np.float32)).astype(np.float32)

    minv_stacks, rho_to_idx, iter_minv_idx = [], {}, []
    for r in rhos:
        key = float(r)
        if key not in rho_to_idx:
            M = np.linalg.inv(AAH + key * np.eye(Nv))
            Mr = M.real.astype(np.float32)
            Mi = M.imag.astype(np.float32)
            mstk = np.concatenate([Mr.T, Mi.T], 0).reshape(8, 128, 512)
            mstk = mstk[[0, 4, 1, 5, 2, 6, 3, 7]]
            minv_stacks.append(
                mstk.transpose(1, 0, 2).copy().astype(ml_dtypes.bfloat16))
            rho_to_idx[key] = len(minv_stacks) - 1
        iter_minv_idx.append(rho_to_idx[float(r)])
    nu = len(minv_stacks)

    A1 = np.concatenate([Ar, Ai], 0)                    # [1024, 2048]
    AB = A1.reshape(8, 128, 2048)[[0, 4, 1, 5, 2, 6, 3, 7]]
    AB = AB.transpose(1, 0, 2).copy().astype(ml_dtypes.bfloat16)
    AT1 = np.concatenate([Ar.T, Ai.T], 0)               # [4096, 512]
    ATR = AT1.reshape(32, 128, 512)
    ilv = [x for tc in range(16) for x in (tc, 16 + tc)]
    ATR = ATR[ilv].transpose(1, 0, 2).copy().astype(ml_dtypes.bfloat16)

    # bf16 CNN weights pack [128, 896]
    w1 = np.asarray(inputs['conv1_w'], np.float32)
    w2 = np.asarray(inputs['conv2_w'], np.float32)
    wf = np.asarray(inputs['convf_w'], np.float32)
    W1 = np.zeros((128, 128), np.float32)
    for dl in range(3):
        for ci in range(2):
            for q in range(4):
                W1[dl * 8 + ci * 4 + q, np.arange(F) * 4 + q] = w1[:, ci, dl]
    W2 = np.zeros((3, 128, 128), np.float32)
    for dl in range(3):
        for ci in range(F):
            for q in range(4):
                W2[dl, ci * 4 + q, np.arange(F) * 4 + q] = w2[:, ci, dl]
    WFb = np.zeros((4, 3, 128, 32), np.float32)
    for bp in range(4):
        for dl in range(3):
            for ci in range(F):
                for q in range(4):
                    for cf in range(2):
                        WFb[bp, dl, ci * 4 + q, cf * 16 + bp * 4 + q] = wf[cf, ci, dl]
    WTB = np.concatenate(
        [W1] + [W2[d] for d in range(3)]
        + [WFb[bp, dl] for bp in range(4) for dl in range(3)],
        axis=1).astype(ml_dtypes.bfloat16)              # [128, 896]

    # f32 selector/identity pack
    OSEL = np.zeros((128, 32), np.float32)
    SELB = np.zeros((128, 128), np.float32)   # rows 0-31 used
    for co in range(32):
        for q in range(4):
            OSEL[co * 4 + q, co] = 1.0
            SELB[co, co * 4 + q] = 1.0
    I16 = np.eye(16, dtype=np.float32)
    JN = np.zeros((32, 32), np.float32)
    JN[16:, :16] = -I16
    JN[:16, 16:] = I16
    JH = -JN
    ID32 = np.eye(32, dtype=np.float32)
    IDJN = np.zeros((128, 64), np.float32)
    IDJN[:32, :32] = ID32
    IDJN[:32, 32:] = JN
    IDJH = np.zeros((128, 64), np.float32)
    IDJH[:32, :32] = ID32
    IDJH[:32, 32:] = JH
    P32 = np.zeros((128, 32), np.float32)
    for m in range(32):
        P32[m, m] = 1.0
        P32[(m + 16) % 32, m] = 1.0
    ID32p = np.zeros((128, 32), np.float32)
    ID32p[:32, :32] = ID32
    WTS = np.concatenate([OSEL, SELB, IDJN, IDJH, P32, ID32p], axis=1)

    g1 = np.asarray(inputs['bn1_g'], np.float32)
    b1 = np.asarray(inputs['bn1_b'], np.float32)
    g2 = np.asarray(inputs['bn2_g'], np.float32)
    b2 = np.asarray(inputs['bn2_b'], np.float32)
    fb = np.asarray(inputs['convf_b'], np.float32)
    CF = np.zeros((128, 8), np.float32)
    CF[:32, 0] = g1
    CF[:32, 1] = b1
    CF[:32, 2] = g2
    CF[:32, 3] = b2
    CF[:, 6] = BN_EPS
    FBC = np.zeros((128, nu), np.float32)
    for key, idx in rho_to_idx.items():
        c1 = 1.0 / (key + 1e-8)
        FBC[:16, idx] = fb[0] * c1
        FBC[16:32, idx] = fb[1] * c1

    y = np.asarray(inputs['y'], np.float32)
    ybm_cores, ylh_cores, yln_cores = [], [], []
    for c in range(NCORE):
        ys = y[c * BS:(c + 1) * BS]
        ybm = np.concatenate([ys[:, 0], ys[:, 1]], 0)    # [32, Nv]
        ybm_cores.append(np.ascontiguousarray(ybm))
        sT = ybm.T                                       # [Nv, 32]
        botH = np.concatenate([sT[:, 16:], -sT[:, :16]], 1)
        botN = np.concatenate([-sT[:, 16:], sT[:, :16]], 1)
        comboH = np.concatenate([sT, botH], 0)           # [2Nv, 32]
        comboN = np.concatenate([sT, botN], 0)
        ylh_cores.append(comboH.reshape(8, 128, 32)[[0, 4, 1, 5, 2, 6, 3, 7]]
                         .transpose(1, 0, 2).copy().astype(ml_dtypes.bfloat16))
        yln_cores.append(comboN.reshape(8, 128, 32)[[0, 4, 1, 5, 2, 6, 3, 7]]
                         .transpose(1, 0, 2).copy().astype(ml_dtypes.bfloat16))

    return dict(AB=AB, ATR=ATR, minv_stacks=minv_stacks,
                iter_minv_idx=iter_minv_idx, rhos=rhos, epss=epss,
                WTS=WTS, WTB=WTB, CF=CF, FBC=FBC, ybm_cores=ybm_cores,
                ylh_cores=ylh_cores, yln_cores=yln_cores)


# WTS column offsets
OSEL_C = 0
SELB_C = 32
IDJN_C = SELB_C + 128
IDJH_C = IDJN_C + 64
P32_C = IDJH_C + 64
ID32_C = P32_C + 32
WTS_W = ID32_C + 32
# WTB column offsets
W1_C = 0
W2_C = 128
WF_C = W2_C + 384
WTB_W = WF_C + 384


# ---------------------------------------------------------------- program
def _build_program(prep):
    import concourse.bacc as bacc
    import concourse.tile as tile
    import concourse.mybir as mybir

    dt = mybir.dt
    f32, f32r, bf16 = dt.float32, dt.float32r, dt.bfloat16
    AX = mybir.AxisListType
    OP = mybir.AluOpType
    AF = mybir.ActivationFunctionType

    nu = len(prep['minv_stacks'])
    rhos, epss = prep['rhos'], prep['epss']
    cnt = float(B * Nt)

    nc = bacc.Bacc("TRN2", target_bir_lowering=False, debug=False,
                   num_devices=NCORE)

    AB_d = nc.dram_tensor("AB", [128, 8, 2048], bf16, kind="ExternalInput")
    ATR_d = nc.dram_tensor("ATR", [128, 32, 512], bf16, kind="ExternalInput")
    MINV_d = nc.dram_tensor("MINVS", [nu, 128, 8, 512], bf16, kind="ExternalInput")
    WTS_d = nc.dram_tensor("WTS", [128, WTS_W], f32r, kind="ExternalInput")
    WTB_d = nc.dram_tensor("WTB", [128, WTB_W], bf16, kind="ExternalInput")
    CF_d = nc.dram_tensor("CF", [128, 8], f32, kind="ExternalInput")
    FBC_d = nc.dram_tensor("FBC", [128, nu], f32, kind="ExternalInput")
    Y_d = nc.dram_tensor("YBM", [32, 512], f32r, kind="ExternalInput")
    YLH_d = nc.dram_tensor("YLH", [128, 8, 32], bf16, kind="ExternalInput")
    YLN_d = nc.dram_tensor("YLN", [128, 8, 32], bf16, kind="ExternalInput")
    XO_d = nc.dram_tensor("XOUT", [32, 2048], f32r, kind="ExternalOutput")

    with tile.TileContext(nc) as tc:
        with (
            tc.tile_pool(name="cst", bufs=1) as cst,
            tc.tile_pool(name="st", bufs=1) as stp,
            tc.tile_pool(name="cnn", bufs=1) as cnnp,
            tc.tile_pool(name="xin", bufs=4) as xinp,
            tc.tile_pool(name="psbig", bufs=2, space="PSUM") as psbig,
            tc.tile_pool(name="psgen", bufs=2, space="PSUM") as psgen,
            tc.tile_pool(name="psarc", bufs=1, space="PSUM") as psarc,
            tc.tile_pool(name="pssm", bufs=3, space="PSUM") as pssm,
            tc.tile_pool(name="ddr", bufs=2, space="DRAM") as ddr,
        ):
            # ---- constants ----
            ab = cst.tile([128, 8, 2048], bf16, tag="ab")
            atr = cst.tile([128, 32, 512], bf16, tag="atr")
            minv = cst.tile([128, 8, 512], bf16, tag="minv")
            wts = cst.tile([128, WTS_W], f32r, tag="wts")
            wtb = cst.tile([128, WTB_W], bf16, tag="wtb")
            cf = cst.tile([128, 8], f32, tag="cf")
            fbc = cst.tile([128, nu], f32, tag="fbc")
            ylh = cst.tile([128, 8, 32], bf16, tag="ylh")
            yln = cst.tile([128, 8, 32], bf16, tag="yln")
            nc.sync.dma_start(ab[:], AB_d[:])
            nc.sync.dma_start(atr[:], ATR_d[:])
            nc.sync.dma_start(wts[:], WTS_d[:])
            nc.sync.dma_start(wtb[:], WTB_d[:])
            nc.sync.dma_start(cf[:], CF_d[:])
            nc.sync.dma_start(fbc[:], FBC_d[:])
            nc.sync.dma_start(ylh[:], YLH_d[:])
            nc.sync.dma_start(yln[:], YLN_d[:])
            if nu == 1:
                nc.sync.dma_start(minv[:], MINV_d[0])

            OSEL = wts[:, OSEL_C:OSEL_C + 32]
            SELB = wts[0:32, SELB_C:SELB_C + 128]
            IDJN = wts[0:32, IDJN_C:IDJN_C + 64]
            IDJH = wts[0:32, IDJH_C:IDJH_C + 64]
            P32 = wts[0:32, P32_C:P32_C + 32]
            ID32 = wts[0:32, ID32_C:ID32_C + 32]
            W1 = wtb[0:24, W1_C:W1_C + 128]
            W2 = [wtb[:, W2_C + 128 * d: W2_C + 128 * (d + 1)] for d in range(3)]
            WF = [[wtb[:, WF_C + (bp * 3 + d) * 32: WF_C + (bp * 3 + d) * 32 + 32]
                   for d in range(3)] for bp in range(4)]
            g32 = [cf[0:32, 0:1], cf[0:32, 2:3]]
            b32 = [cf[0:32, 1:2], cf[0:32, 3:4]]
            zb128 = cf[:, 5:6]
            zb32 = cf[0:32, 5:6]
            epsb = cf[0:32, 6:7]

            # ---- state ----
            X = stp.tile([32, 2048], f32r, tag="X")
            RC = stp.tile([32, 2048], f32r, tag="RC")
            XB = stp.tile([32, 2050], bf16, tag="XB")
            S = stp.tile([32, 8, 512], f32r, tag="S")
            y_t, u_t, zmu_t, marc_t = S[:, 0, :], S[:, 1, :], S[:, 2, :], S[:, 3, :]
            q3_t, t1_t, dif_t, sq_t = S[:, 4, :], S[:, 5, :], S[:, 6, :], S[:, 7, :]
            arcS = stp.tile([32, 512], f32r, tag="arcS")
            mzS = stp.tile([32, 512], f32r, tag="mzS")
            s32f = stp.tile([32, 1], f32, tag="s32f")
            facs = stp.tile([32, 4], f32, tag="facs")
            smal = stp.tile([32, 8], f32r, tag="smal")
            gbt = stp.tile([32, 2], f32r, tag="gbt")
            gbb = stp.tile([128, 2], f32, tag="gbb")
            zmuT = stp.tile([128, 8, 32], bf16, tag="zmuT")
            wT = stp.tile([128, 8, 32], bf16, tag="wT")
            rcT = stp.tile([128, 32, 32], bf16, tag="rcT")
            stat = cnnp.tile([128, 32], f32, tag="stat")
            stat2 = cnnp.tile([128, 2], f32, tag="stat2")
            stat2r = cnnp.tile([128, 2], f32r, tag="stat2r")
            p32s = cnnp.tile([32, 2], f32r, tag="p32s")
            sdma = cnnp.tile([2, 32], f32, tag="sdma")
            st2 = cnnp.tile([2, 32], f32r, tag="st2")
            agg = cnnp.tile([2, 8, 32], f32, tag="agg")

            A1 = cnnp.tile([128, 4, 2050], bf16, tag="A1")
            A2 = cnnp.tile([128, 4, 2050], bf16, tag="A2")
            SQD = cnnp.tile([128, 2, 512], bf16, tag="SQD")

            nc.sync.dma_start(y_t[:], Y_d[:])
            nc.vector.memset(u_t[:].bitcast(f32), 0.0)
            nc.vector.tensor_copy(zmu_t[:], y_t[:])
            nc.vector.memset(XB[:, 0:1].bitcast(dt.uint16), 0)
            nc.vector.memset(XB[:, 2049:2050].bitcast(dt.uint16), 0)
            for a in (A1, A2):
                nc.vector.memset(a[:, :, 0:1].bitcast(dt.uint16), 0)
                nc.vector.memset(a[:, :, 2049:2050].bitcast(dt.uint16), 0)

            # ---- x0 = A^H y ----
            for ntb in range(4):
                p = psgen.tile([32, 512], f32, tag="mm")
                for k in range(8):
                    nc.tensor.matmul(p[:], ylh[:, k, :],
                                     ab[:, k, 512 * ntb:512 * (ntb + 1)],
                                     start=(k == 0), stop=(k == 7))
                nc.vector.tensor_copy(X[:, 512 * ntb:512 * (ntb + 1)], p[:])

            def combo_pair(dst, pair, src_view, idj):
                # plain matmul: out = src^T @ [I|J] (is_transpose ignores the
                # identity's values, so signed J requires the normal datapath);
                # top/bottom slots adjacent -> single cast copy
                pT = pssm.tile([128, 64], f32, tag="sm")
                nc.tensor.matmul(pT[:], src_view, idj, start=True, stop=True)
                nc.vector.tensor_copy(
                    dst[:, 2 * pair:2 * pair + 2, :],
                    pT[:].rearrange("p (a b) -> p a b", a=2))

            def bn_collective(layer, acts):
                with nc.allow_low_precision(reason="bn stat reduce"):
                    nc.vector.tensor_reduce(stat2[:, 0:1], stat[:, 0:16], AX.X, OP.add)
                    nc.vector.tensor_reduce(stat2[:, 1:2], stat[:, 16:32], AX.X, OP.add)
                nc.vector.tensor_copy(stat2r[:], stat2[:])
                p = pssm.tile([32, 2], f32, tag="sm")
                nc.tensor.matmul(p[:], OSEL, stat2r[:], start=True, stop=True)
                with nc.allow_low_precision(reason="pack"):
                    nc.vector.tensor_copy(p32s[:], p[:])
                pt = pssm.tile([2, 32], f32, tag="sm")
                nc.tensor.matmul(pt[:], p32s[:], ID32, start=True, stop=True)
                nc.vector.tensor_copy(sdma[:], pt[:])
                ci_ = ddr.tile([2, 32], f32, tag="cc")
                if USE_AG:
                    co_ = ddr.tile([8, 2, 32], f32, tag="cc2")
                    nc.sync.dma_start(ci_[:], sdma[:])
                    nc.gpsimd.collective_compute(
                        "AllGather", OP.bypass,
                        replica_groups=[list(range(NCORE))],
                        ins=[ci_.opt()], outs=[co_.opt()])
                    nc.sync.dma_start(agg[:], co_[:].rearrange("c s v -> s c v"))
                    with nc.allow_low_precision(reason="bn stat reduce"):
                        nc.vector.tensor_reduce(
                            st2[:], agg[:].rearrange("s c v -> s v c"),
                            AX.X, OP.add)
                else:
                    co1 = ddr.tile([2, 32], f32, tag="cc3")
                    nc.sync.dma_start(ci_[:], sdma[:])
                    nc.gpsimd.collective_compute(
                        "AllReduce", OP.add,
                        replica_groups=[list(range(NCORE))],
                        ins=[ci_.opt()], outs=[co1.opt()])
                    nc.sync.dma_start(st2[:].bitcast(f32), co1[:])
                pb = pssm.tile([32, 32], f32, tag="sm")
                nc.tensor.matmul(pb[:, 0:2], st2[:], ID32[0:2, 0:2],
                                 start=True, stop=True)
                with nc.allow_low_precision(reason="bn scalar math in f32r"):
                    nc.vector.tensor_scalar_mul(smal[:, 0:1], pb[:, 0:1], 1.0 / cnt)
                    nc.vector.tensor_scalar_mul(smal[:, 1:2], pb[:, 1:2], 1.0 / cnt)
                    nc.vector.tensor_mul(smal[:, 2:3], smal[:, 0:1], smal[:, 0:1])
                    nc.vector.tensor_sub(smal[:, 3:4], smal[:, 1:2], smal[:, 2:3])
                    nc.scalar.activation(smal[:, 3:4], smal[:, 3:4], AF.Sqrt,
                                         bias=epsb)
                    nc.vector.reciprocal(smal[:, 3:4], smal[:, 3:4])
                    nc.vector.tensor_mul(gbt[:, 0:1], g32[layer], smal[:, 3:4])
                    nc.vector.tensor_mul(smal[:, 2:3], smal[:, 0:1], gbt[:, 0:1])
                    nc.vector.tensor_sub(gbt[:, 1:2], b32[layer], smal[:, 2:3])
                p2 = pssm.tile([128, 2], f32, tag="sm")
                nc.tensor.matmul(p2[:], SELB, gbt[:], start=True, stop=True)
                nc.vector.tensor_copy(gbb[:], p2[:])
                for bp in range(2):
                    nc.scalar.activation(acts[:, bp, 1:2049], acts[:, bp, 1:2049],
                                         AF.Relu, bias=gbb[:, 1:2],
                                         scale=gbb[:, 0:1])
                for bp in range(2, 4):
                    av = acts[:, bp, 1:2049]
                    nc.vector.tensor_scalar(av, av, gbb[:, 0:1], gbb[:, 1:2],
                                            OP.mult, OP.add)
                    nc.vector.tensor_scalar(av, av, 0.0, None, OP.max)

            # ================= iterations =================
            for it in range(ITERS):
                rho = float(rhos[it])
                eps = float(epss[it])
                c1 = 1.0 / (rho + 1e-8)
                uidx = prep['iter_minv_idx'][it]
                if nu > 1:
                    nc.sync.dma_start(minv[:], MINV_d[uidx])

                # ---- CNN: xb cast + gather ----
                nc.vector.tensor_copy(XB[:, 1:2049], X[:])
                xins = []
                for bp in range(4):
                    xin = xinp.tile([24, 2048], bf16, tag="xin")
                    for dl in range(3):
                        for ci in range(2):
                            eng = nc.sync if (dl % 2 == 0) else nc.scalar
                            eng.dma_start(
                                xin[dl * 8 + ci * 4: dl * 8 + ci * 4 + 4, :],
                                XB[ci * 16 + bp * 4: ci * 16 + bp * 4 + 4,
                                   dl:dl + 2048])
                    xins.append(xin)
                # conv1
                for lt in range(4):
                    for bp in range(4):
                        pb_ = psbig.tile([128, 512], f32, tag="big")
                        nc.tensor.matmul(
                            pb_[:], W1,
                            xins[bp][0:24, 512 * lt:512 * (lt + 1)],
                            start=True, stop=True)
                        g = lt * 4 + bp
                        a1v = A1[:, bp, 1 + 512 * lt:1 + 512 * (lt + 1)]
                        nc.scalar.activation(a1v, pb_[:], AF.Identity,
                                             bias=zb128,
                                             accum_out=stat[:, g:g + 1])
                        nc.vector.scalar_tensor_tensor(
                            SQD[:, 0, :], a1v, 1.0, a1v, OP.mult, OP.mult,
                            accum_out=stat[:, 16 + g:17 + g])

                # step-0 mz: runs on PE during the bn1 collective stall
                lhs0 = yln if it == 0 else zmuT
                pmz0 = psgen.tile([32, 512], f32, tag="mm")
                for k in range(8):
                    nc.tensor.matmul(pmz0[:], lhs0[:, k, :], minv[:, k, :],
                                     start=(k == 0), stop=(k == 7))
                nc.vector.tensor_copy(mzS[:], pmz0[:])

                bn_collective(0, A1)

                # conv2
                for lt in range(4):
                    for bp in range(4):
                        pb_ = psbig.tile([128, 512], f32, tag="big")
                        for dl in range(3):
                            nc.tensor.matmul(
                                pb_[:], W2[dl],
                                A1[:, bp, dl + 512 * lt: dl + 512 * lt + 512],
                                start=(dl == 0), stop=(dl == 2))
                        g = lt * 4 + bp
                        a2v = A2[:, bp, 1 + 512 * lt:1 + 512 * (lt + 1)]
                        nc.scalar.activation(a2v, pb_[:], AF.Identity,
                                             bias=zb128,
                                             accum_out=stat[:, g:g + 1])
                        nc.vector.scalar_tensor_tensor(
                            SQD[:, 0, :], a2v, 1.0, a2v, OP.mult, OP.mult,
                            accum_out=stat[:, 16 + g:17 + g])
                bn_collective(1, A2)

                # convf + rc1 + comboN(rc1) + arc matmuls
                parc = psarc.tile([32, 512], f32, tag="arc")
                first_mm = True
                for lt in range(4):
                    pcf = psgen.tile([32, 512], f32, tag="mm")
                    nmm = 0
                    for bp in range(4):
                        for dl in range(3):
                            nc.tensor.matmul(
                                pcf[:], WF[bp][dl],
                                A2[:, bp, dl + 512 * lt: dl + 512 * lt + 512],
                                start=(nmm == 0), stop=(nmm == 11))
                            nmm += 1
                    blk = slice(512 * lt, 512 * (lt + 1))
                    # rc1 = (pcf + x)*c1 + fb*c1
                    nc.vector.scalar_tensor_tensor(
                        sq_t[:], pcf[:], 1.0, X[:, blk], OP.mult, OP.add)
                    nc.scalar.activation(RC[:, blk], sq_t[:], AF.Identity,
                                         bias=fbc[0:32, uidx:uidx + 1], scale=c1)
                    for c in range(4):
                        combo_pair(rcT, lt * 4 + c,
                                   RC[:, 512 * lt + 128 * c:
                                      512 * lt + 128 * (c + 1)], IDJN)
                    for c in range(4):
                        tc_ = lt * 4 + c
                        nc.tensor.matmul(parc[:], rcT[:, 2 * tc_, :],
                                         atr[:, 2 * tc_, :],
                                         start=first_mm, stop=False)
                        first_mm = False
                        last = (lt == 3 and c == 3)
                        nc.tensor.matmul(parc[:], rcT[:, 2 * tc_ + 1, :],
                                         atr[:, 2 * tc_ + 1, :],
                                         start=False, stop=last)
                nc.vector.tensor_copy(arcS[:], parc[:])
                # comboN(arc) -> wT scratch; marc = Minv @ arc
                for c in range(4):
                    combo_pair(wT, c, arcS[:, 128 * c:128 * (c + 1)], IDJN)
                pma = psgen.tile([32, 512], f32, tag="mm")
                for k in range(8):
                    nc.tensor.matmul(pma[:], wT[:, k, :], minv[:, k, :],
                                     start=(k == 0), stop=(k == 7))
                nc.vector.tensor_copy(marc_t[:], pma[:])

                # ---------- ADMM steps ----------
                r2c1 = rho * rho * c1
                r3c1 = rho * rho * rho * c1
                for s in range(ADMM):
                    final = (s == ADMM - 1)
                    nc.vector.scalar_tensor_tensor(
                        t1_t[:], zmu_t[:], r2c1, u_t[:], OP.mult, OP.add)
                    nc.vector.scalar_tensor_tensor(
                        t1_t[:], marc_t[:], rho, t1_t[:], OP.mult, OP.add)
                    nc.vector.tensor_sub(q3_t[:], t1_t[:], y_t[:])
                    if s == 0:
                        mz_ap = mzS[:]
                    else:
                        pmz = psgen.tile([32, 512], f32, tag="mm")
                        for k in range(8):
                            nc.tensor.matmul(pmz[:], zmuT[:, k, :], minv[:, k, :],
                                             start=(k == 0), stop=(k == 7))
                        mz_ap = pmz[:]
                    nc.vector.scalar_tensor_tensor(
                        dif_t[:], mz_ap, -r3c1, q3_t[:], OP.mult, OP.add)
                    if final:
                        nc.vector.scalar_tensor_tensor(
                            t1_t[:], mz_ap, r2c1, marc_t[:], OP.mult, OP.subtract)
                    nc.vector.scalar_tensor_tensor(
                        sq_t[:], dif_t[:], 1.0, dif_t[:], OP.mult, OP.mult,
                        accum_out=s32f[:])
                    pn = pssm.tile([32, 1], f32, tag="sm")
                    nc.tensor.matmul(pn[:], P32.bitcast(f32), s32f[:],
                                     start=True, stop=True)
                    nc.scalar.activation(facs[:, 0:1], pn[:], AF.Sqrt, bias=zb32)
                    nc.vector.reciprocal(facs[:, 0:1], facs[:, 0:1])
                    nc.vector.tensor_scalar(facs[:, 1:2], facs[:, 0:1],
                                            eps, 1.0, OP.mult, OP.min)
                    nc.vector.tensor_scalar(facs[:, 2:3], facs[:, 1:2],
                                            -1.0, 1.0, OP.mult, OP.add)
                    if final:
                        nc.vector.tensor_scalar(facs[:, 3:4], facs[:, 1:2],
                                                1.0, -1.0, OP.mult, OP.add)
                    else:
                        nc.vector.tensor_scalar(facs[:, 3:4], facs[:, 1:2],
                                                2.0, -1.0, OP.mult, OP.add)
                    last_all = (it == ITERS - 1 and final)
                    if not last_all:
                        nc.vector.scalar_tensor_tensor(
                            zmu_t[:], dif_t[:], facs[:, 3:4], y_t[:],
                            OP.mult, OP.add)
                        nc.vector.tensor_scalar(u_t[:], dif_t[:], facs[:, 2:3],
                                                None, OP.mult)
                        for c in range(4):
                            combo_pair(zmuT, c,
                                       zmu_t[:, 128 * c:128 * (c + 1)], IDJN)
                    if final:
                        for c in range(4):
                            combo_pair(wT, c,
                                       t1_t[:, 128 * c:128 * (c + 1)], IDJH)
                        for ntb in range(4):
                            px = psgen.tile([32, 512], f32, tag="mm")
                            for k in range(8):
                                nc.tensor.matmul(
                                    px[:], wT[:, k, :],
                                    ab[:, k, 512 * ntb:512 * (ntb + 1)],
                                    start=(k == 0), stop=(k == 7))
                            blk = slice(512 * ntb, 512 * (ntb + 1))
                            nc.vector.tensor_add(X[:, blk], px[:], RC[:, blk])

            nc.sync.dma_start(XO_d[:], X[:])

    nc.compile()
    return nc


def _enable_trace_shim():
    import sys, types
    try:
        import trn_agent_boot.trn_boot as _tb
        import concourse.bass_utils as _bu
        _bu.upload_artifacts = lambda tmpdir: "local://" + str(tmpdir)
        hookmod = types.ModuleType('antenv.axon_hooks')
        hook = _tb._ntff_profile_via_ctypes('/opt/axon/libaxon_pjrt.so')
        hookmod.get_axon_ntff_profile_hook = lambda: hook
        import antenv as _antenv
        sys.modules['antenv.axon_hooks'] = hookmod
        _antenv.axon_hooks = hookmod
        return True
    except Exception:
        return False


def kernel(**inputs) -> np.ndarray:
    import os
    from concourse.bass_utils import run_bass_kernel_spmd
    trace = bool(os.environ.get("KERNEL_TRACE"))
    if trace:
        trace = _enable_trace_shim()

    prep = _host_prep(inputs)
    nc = _build_program(prep)

    minvs = np.stack(prep['minv_stacks'], 0)
    in_maps = []
    for c in range(NCORE):
        in_maps.append({
            "AB": prep['AB'], "ATR": prep['ATR'], "MINVS": minvs,
            "WTS": prep['WTS'], "WTB": prep['WTB'], "CF": prep['CF'],
            "FBC": prep['FBC'],
            "YBM": np.ascontiguousarray(prep['ybm_cores'][c][:, :512]),
            "YLH": prep['ylh_cores'][c], "YLN": prep['yln_cores'][c],
        })
    res = run_bass_kernel_spmd(nc, in_maps, list(range(NCORE)), trace=trace)
    out = np.zeros((B, 2, Nt), np.float32)
    for c in range(NCORE):
        xc = res.results[c]["XOUT"]
        out[c * BS:(c + 1) * BS, 0] = xc[:16]
        out[c * BS:(c + 1) * BS, 1] = xc[16:]
    kernel._last_results = res
    return out
